# revision 1
# baseline (speedup 1.0000x reference)
"""Bass/Trainium2 kernel for batched masked-Kabsch RMSD (nn_Coords2RMSD).

Strategy (per NeuronCore, SPMD across 8 cores):
  - Host sorts batch rows by num_atoms and forms 4 size classes (quartiles
    of the sorted order). Core c takes one 128-row tile from each class;
    class k is processed with a fixed atom capacity cap[k] (max natoms in
    the class, rounded up), so cores run identical programs while skipping
    the padded tail of short rows.
  - Per tile: SWDGE DMA cast-loads the f32 coords to bf16 SBUF, DVE builds
    the atom mask and de-interleaves xyz with the mask multiply, then 9
    scalar_tensor_tensor products with fused fp32 accumulation produce the
    raw second moments; ScalarE accumulates Sx/Sy/|x|^2/|y|^2.
  - Final stage (tiny [128, 4] fp32 tiles): centroid corrections, 3x3
    C^T C eigenvalues via the closed-form trigonometric method (acos built
    from Arctan+Sqrt, cos via Sin with phase bias), Kabsch det sign, RMSD.
"""

import numpy as np

import concourse.bass as bass
import concourse.mybir as mybir
from concourse.tile import TileContext, ScopedClock

F32 = mybir.dt.float32
BF16 = mybir.dt.bfloat16
OP = mybir.AluOpType
AF = mybir.ActivationFunctionType

N_CORES = 8
ROWS = 128  # rows per tile == SBUF partitions


# ---------------------------------------------------------------------------
# TileContext tail patch: this walrus build accepts at most ONE sync-wait
# command per instruction and no sem-eq waits, so the stock drain + EVSEM
# butterfly fails codegen. Emit a ge-wait-only tail instead.
# ---------------------------------------------------------------------------
def _patched_drain_and_barrier(self, tick_clock, wait_clock):
    nc = self.nc
    dummy = nc.gpsimd.nop()
    wait_clock.add_sem_waits(dummy.ins, ScopedClock({None: tick_clock.global_clock}))
    waits = list(dummy.ins.sync_info.on_wait) if dummy.ins.sync_info else []
    if dummy.ins.sync_info:
        dummy.ins.sync_info = mybir.SyncInfo(on_wait=[], on_update=[])

    bsem = nc.alloc_semaphore(f"tail_bsem_{nc.next_id()}")
    dsem = nc.alloc_semaphore(f"tail_dsem_{nc.next_id()}")
    n_eng = 0
    for eng in nc.engines.values():
        eng.drain()
        eng.sem_inc(bsem, 1)
        n_eng += 1
    # gpsimd observes every engine and every outstanding work/DMA sem, then
    # broadcasts that knowledge via dsem so the range-clear happens-after
    # everything on every engine.
    nc.gpsimd.wait_ge(bsem, n_eng)
    for w in waits:
        n = nc.gpsimd.nop()
        n.ins.sync_info = mybir.SyncInfo(on_wait=[w], on_update=[])
    nc.gpsimd.sem_inc(dsem, 1)
    for eng in nc.engines.values():
        if eng is not nc.gpsimd:
            eng.wait_ge(dsem, 1)

    popped = nc._tile_sem_poison_stack.pop()
    assert popped is self._sem_poison
    nc.clear_and_free_semaphores(list(self.sems.allocated().values()))
    nc.gpsimd.sem_clear(bsem)
    nc.gpsimd.sem_clear(dsem)


def install_tile_patch():
    TileContext._drain_and_barrier = _patched_drain_and_barrier


# ---------------------------------------------------------------------------
# BIR post-pass: this walrus build accepts at most one sync-wait command per
# instruction (none on Drain). Tile's sem-assigner can attach several, so
# split extras onto same-engine NoOps inserted just before the instruction.
# ---------------------------------------------------------------------------
_orig_to_json_bytes = bass.Bass.to_json_bytes


def _split_multiwait_json(self) -> bytes:
    import json

    raw = _orig_to_json_bytes(self)
    m = json.loads(raw)
    ctr = 0
    changed = False
    for f in m.get("functions", []):
        for blk in f.get("blocks", []):
            insts = blk.get("instructions", [])
            out = []
            for inst in insts:
                si = inst.get("sync_info")
                ow = (si or {}).get("on_wait") or []
                opc = str(inst.get("opcode", inst.get("type", "")))
                limit = 0 if opc == "Drain" else 1
                if len(ow) > limit:
                    keep = ow[len(ow) - limit :] if limit else []
                    moved = ow[: len(ow) - limit] if limit else ow
                    for w in moved:
                        ctr += 1
                        out.append(
                            {
                                "debug": inst.get("debug", 0),
                                "engine": inst["engine"],
                                "ins": [],
                                "name": f"WS-{ctr}-{inst['name']}",
                                "opcode": "NoOp",
                                "outs": [],
                                "sync_info": {"on_update": [], "on_wait": [w]},
                            }
                        )
                    si["on_wait"] = keep
                    changed = True
                out.append(inst)
            blk["instructions"] = out
    if not changed:
        return raw
    return json.dumps(m).encode()


bass.Bass.to_json_bytes = _split_multiwait_json


# ---------------------------------------------------------------------------
# Final math emitter: everything on [128, K] fp32 tiles.
# ---------------------------------------------------------------------------
class _FM:
    def __init__(self, nc, pool, K):
        self.nc = nc
        self.pool = pool
        self.K = K
        self.n = 0
        self._consts = {}

    def const_col(self, val):
        val = float(val)
        if val in self._consts:
            return self._consts[val]
        i = len(self._consts)
        t = self.pool.tile([ROWS, 1], F32, tag=f"fmc{i}", name=f"fmc{i}")
        self.nc.vector.memset(t[:], val)
        self._consts[val] = t[:]
        return t[:]

    def t(self):
        self.n += 1
        return self.pool.tile([ROWS, self.K], F32, tag=f"fm{self.n}", name=f"fm{self.n}")

    def tt(self, a, b, op):
        o = self.t()
        self.nc.vector.tensor_tensor(o[:], a, b, op)
        return o[:]

    def mul(self, a, b):
        return self.tt(a, b, OP.mult)

    def add(self, a, b):
        return self.tt(a, b, OP.add)

    def sub(self, a, b):
        return self.tt(a, b, OP.subtract)

    def ts(self, a, s, op):
        o = self.t()
        self.nc.vector.tensor_scalar(o[:], a, float(s), None, op)
        return o[:]

    def ts2(self, a, s1, s2, op0, op1):
        o = self.t()
        self.nc.vector.tensor_scalar(o[:], a, float(s1), float(s2), op0, op1)
        return o[:]

    def stt(self, a, s, b, op0, op1):
        """(a op0 s) op1 b"""
        o = self.t()
        self.nc.vector.scalar_tensor_tensor(o[:], a, float(s), b, op0, op1)
        return o[:]

    def act(self, a, func, bias=0.0, scale=1.0):
        o = self.t()
        if isinstance(bias, float) and bias not in (0.0, 1.0) and func != AF.Copy:
            bias = self.const_col(bias)
        self.nc.scalar.activation(o[:], a, func, bias=bias, scale=scale)
        return o[:]

    def recip(self, a):
        o = self.t()
        self.nc.vector.reciprocal(o[:], a)
        return o[:]


def _emit_final_math(nc, fm, st_rxy, st_g, st_s, meta_t, out_ap, K):
    n_ap = meta_t[:]
    rn = fm.recip(n_ap)

    def Sx(i):
        return st_s[:, i : 6 * K : 6]

    def Sy(j):
        return st_s[:, 3 + j : 6 * K : 6]

    def Rxy(i, j):
        return st_rxy[:, 3 * i + j : 9 * K : 9]

    # C_ij = Rxy_ij - Sx_i * Sy_j * rn
    C = [[None] * 3 for _ in range(3)]
    for i in range(3):
        for j in range(3):
            t = fm.mul(Sx(i), Sy(j))
            C[i][j] = fm.sub(Rxy(i, j), fm.mul(t, rn))

    # gx = Rxx - (|Sx|^2) rn ; gy = Ryy - (|Sy|^2) rn
    sx2 = fm.add(fm.add(fm.mul(Sx(0), Sx(0)), fm.mul(Sx(1), Sx(1))), fm.mul(Sx(2), Sx(2)))
    sy2 = fm.add(fm.add(fm.mul(Sy(0), Sy(0)), fm.mul(Sy(1), Sy(1))), fm.mul(Sy(2), Sy(2)))
    gx = fm.sub(st_g[:, 0 : 2 * K : 2], fm.mul(sx2, rn))
    gy = fm.sub(st_g[:, 1 : 2 * K : 2], fm.mul(sy2, rn))

    # K = C^T C (symmetric; k[a][b] = sum_i C[i][a] C[i][b])
    kk = {}
    for a in range(3):
        for b in range(a, 3):
            s = fm.mul(C[0][a], C[0][b])
            s = fm.add(s, fm.mul(C[1][a], C[1][b]))
            s = fm.add(s, fm.mul(C[2][a], C[2][b]))
            kk[(a, b)] = s

    # det(C)
    m0 = fm.sub(fm.mul(C[1][1], C[2][2]), fm.mul(C[1][2], C[2][1]))
    m1 = fm.sub(fm.mul(C[1][0], C[2][2]), fm.mul(C[1][2], C[2][0]))
    m2 = fm.sub(fm.mul(C[1][0], C[2][1]), fm.mul(C[1][1], C[2][0]))
    detC = fm.add(fm.sub(fm.mul(C[0][0], m0), fm.mul(C[0][1], m1)), fm.mul(C[0][2], m2))

    # q = tr(K)/3 ; p2 = sum (k_aa - q)^2 + 2 (k01^2 + k02^2 + k12^2)
    q = fm.ts(fm.add(fm.add(kk[(0, 0)], kk[(1, 1)]), kk[(2, 2)]), 1.0 / 3.0, OP.mult)
    kd = [fm.sub(kk[(a, a)], q) for a in range(3)]
    p2 = fm.add(fm.add(fm.mul(kd[0], kd[0]), fm.mul(kd[1], kd[1])), fm.mul(kd[2], kd[2]))
    xsq = fm.add(
        fm.add(fm.mul(kk[(0, 1)], kk[(0, 1)]), fm.mul(kk[(0, 2)], kk[(0, 2)])),
        fm.mul(kk[(1, 2)], kk[(1, 2)]),
    )
    p2 = fm.stt(xsq, 2.0, p2, OP.mult, OP.add)  # p2 + 2*xsq
    # p = sqrt(max(p2/6, tiny))
    p2c = fm.ts(fm.ts(p2, 1.0 / 6.0, OP.mult), 1e-30, OP.max)
    p = fm.act(p2c, AF.Sqrt)

    # det(K - qI) (symmetric)
    k01, k02, k12 = kk[(0, 1)], kk[(0, 2)], kk[(1, 2)]
    d0 = fm.mul(kd[0], fm.sub(fm.mul(kd[1], kd[2]), fm.mul(k12, k12)))
    d1 = fm.mul(k01, fm.sub(fm.mul(k01, kd[2]), fm.mul(k12, k02)))
    d2 = fm.mul(k02, fm.sub(fm.mul(k01, k12), fm.mul(kd[1], k02)))
    detKq = fm.add(fm.sub(d0, d1), d2)

    # r = 0.5 det(K-qI) / p^3, clamped to [-1, 1]
    rp = fm.recip(p)
    r = fm.mul(fm.mul(fm.ts(detKq, 0.5, OP.mult), rp), fm.mul(rp, rp))
    r = fm.ts(fm.ts(r, 1.0, OP.min), -1.0, OP.max)

    # acos via |r| fold (ScalarE Arctan only accepts [-pi/2, pi/2]):
    #   A = 2 atan(sqrt((1-|r|)/(1+|r|))) = acos(|r|), arg in [0, 1]
    #   acos(r) = A + (r<0) * (pi - 2A) ; phi = acos(r)/3
    rabs = fm.stt(r, -1.0, r, OP.mult, OP.max)  # |r| = max(-r, r)
    onemr = fm.act(rabs, AF.Identity, bias=1.0, scale=-1.0)  # 1 - |r|
    onepr = fm.ts(rabs, 1.0, OP.add)
    u = fm.mul(onemr, fm.recip(onepr))
    su = fm.act(u, AF.Sqrt)
    at = fm.act(su, AF.Arctan)
    A = fm.ts(at, 2.0, OP.mult)
    rneg = fm.ts(r, 0.0, OP.is_lt)
    corr = fm.ts2(A, -2.0, float(np.pi), OP.mult, OP.add)  # pi - 2A
    acr = fm.add(A, fm.mul(rneg, corr))
    # cos(phi) = sin(phi + pi/2), arg in [pi/2, pi/2+pi/3] ; 
    # cos(phi + 2pi/3) = -sin(5pi/6 - phi), arg in [pi/2, 5pi/6]
    c1 = fm.act(acr, AF.Sin, bias=float(np.pi / 2), scale=1.0 / 3.0)
    c3m = fm.act(acr, AF.Sin, bias=float(5 * np.pi / 6), scale=-1.0 / 3.0)

    # eigenvalues (l3 = q - 2p*c3m)
    p2x = fm.ts(p, 2.0, OP.mult)
    l1 = fm.add(q, fm.mul(p2x, c1))
    l3 = fm.sub(q, fm.mul(p2x, c3m))
    l2 = fm.sub(fm.stt(q, 3.0, l1, OP.mult, OP.subtract), l3)  # 3q - l1 - l3

    s1 = fm.act(fm.ts(l1, 0.0, OP.max), AF.Sqrt)
    s2 = fm.act(fm.ts(l2, 0.0, OP.max), AF.Sqrt)
    s3 = fm.act(fm.ts(l3, 0.0, OP.max), AF.Sqrt)

    # d = +1 if detC >= 0 else -1  ->  d = 1 - 2*(detC < 0)
    neg = fm.ts(detC, 0.0, OP.is_lt)
    d = fm.act(neg, AF.Identity, bias=1.0, scale=-2.0)

    tr = fm.add(fm.add(s1, s2), fm.mul(d, s3))

    # msd = (gx + gy - 2 tr) rn ; rmsd = sqrt(max(msd, 0))
    diff = fm.stt(tr, -2.0, fm.add(gx, gy), OP.mult, OP.add)
    msd = fm.mul(diff, rn)
    rmsd = fm.act(fm.ts(msd, 0.0, OP.max), AF.Sqrt)
    nc.vector.tensor_copy(out_ap, rmsd)


# ---------------------------------------------------------------------------
# Program builder
# ---------------------------------------------------------------------------
def build_program(caps, nmax, cfg=None):
    """caps: per-class atom capacities (len K). Returns nc."""
    cfg = cfg or {}
    cast_on_dma = cfg.get("cast_on_dma", True)
    dt_main = BF16 if cfg.get("bf16", True) else F32
    K = len(caps)
    capmax = max(caps)
    ncols = 3 * nmax

    install_tile_patch()
    nc = bass.Bass()
    x_d = nc.dram_tensor("x", [K * ROWS, ncols], F32, kind="ExternalInput")
    y_d = nc.dram_tensor("y", [K * ROWS, ncols], F32, kind="ExternalInput")
    iota_d = nc.dram_tensor("iota", [ROWS, nmax], F32, kind="ExternalInput")
    meta_d = nc.dram_tensor("meta", [ROWS, K], F32, kind="ExternalInput")
    out_d = nc.dram_tensor("out", [ROWS, K], F32, kind="ExternalOutput")

    with TileContext(nc) as tc:
        with (
            tc.tile_pool(name="const", bufs=1) as constp,
            tc.tile_pool(name="raw", bufs=cfg.get("raw_bufs", 2)) as rawp,
            tc.tile_pool(name="masked", bufs=cfg.get("masked_bufs", 2)) as mp,
            tc.tile_pool(name="scratch", bufs=1) as scrp,
            tc.tile_pool(name="stats", bufs=1) as statp,
        ):
            iota_t = constp.tile([ROWS, nmax], F32)
            nc.sync.dma_start(out=iota_t[:], in_=iota_d[:])
            meta_t = constp.tile([ROWS, K], F32)
            nc.sync.dma_start(out=meta_t[:], in_=meta_d[:])

            st_rxy = statp.tile([ROWS, 9 * K], F32)
            st_g = statp.tile([ROWS, 2 * K], F32)
            st_s = statp.tile([ROWS, 6 * K], F32)

            for t, cap in enumerate(caps):
                W = 3 * cap
                x_raw = rawp.tile([ROWS, W], dt_main if cast_on_dma else F32, tag="x_raw")
                y_raw = rawp.tile([ROWS, W], dt_main if cast_on_dma else F32, tag="y_raw")
                dma_eng = nc.gpsimd if cast_on_dma else nc.sync
                # rows are component-major on the host side: [x0..xN y0..yN z0..zN]
                x_src = x_d[t * ROWS : (t + 1) * ROWS, :].rearrange(
                    "p (c n) -> p c n", c=3
                )[:, :, 0:cap]
                y_src = y_d[t * ROWS : (t + 1) * ROWS, :].rearrange(
                    "p (c n) -> p c n", c=3
                )[:, :, 0:cap]
                dma_eng.dma_start(out=x_raw[:].rearrange("p (c n) -> p c n", c=3), in_=x_src)
                dma_eng.dma_start(out=y_raw[:].rearrange("p (c n) -> p c n", c=3), in_=y_src)

                m_t = mp.tile([ROWS, cap], dt_main, tag="mask")
                nc.vector.tensor_scalar(
                    m_t[:], iota_t[:, 0:cap], meta_t[:, t : t + 1], None, OP.is_lt
                )

                xm = mp.tile([ROWS, W], dt_main, tag="xm")
                ym = mp.tile([ROWS, W], dt_main, tag="ym")
                for i in range(3):
                    sl = slice(i * cap, (i + 1) * cap)
                    nc.vector.tensor_tensor(xm[:, sl], x_raw[:, sl], m_t[:], OP.mult)
                    nc.vector.tensor_tensor(ym[:, sl], y_raw[:, sl], m_t[:], OP.mult)

                ps = scrp.tile([ROWS, capmax], dt_main, tag="prod")
                for i in range(3):
                    for j in range(3):
                        col = 9 * t + 3 * i + j
                        nc.vector.scalar_tensor_tensor(
                            ps[:, 0:cap],
                            xm[:, i * cap : (i + 1) * cap],
                            1.0,
                            ym[:, j * cap : (j + 1) * cap],
                            OP.mult,
                            OP.mult,
                            accum_out=st_rxy[:, col : col + 1],
                        )

                sq = scrp.tile([ROWS, 3 * capmax], dt_main, tag="sq")
                nc.scalar.activation(
                    sq[:, 0:W], xm[:], AF.Square, accum_out=st_g[:, 2 * t : 2 * t + 1]
                )
                nc.scalar.activation(
                    sq[:, 0:W], ym[:], AF.Square, accum_out=st_g[:, 2 * t + 1 : 2 * t + 2]
                )
                cp = scrp.tile([ROWS, capmax], dt_main, tag="cp")
                for i in range(3):
                    nc.scalar.activation(
                        cp[:, 0:cap],
                        xm[:, i * cap : (i + 1) * cap],
                        AF.Identity,
                        accum_out=st_s[:, 6 * t + i : 6 * t + i + 1],
                    )
                for j in range(3):
                    nc.scalar.activation(
                        cp[:, 0:cap],
                        ym[:, j * cap : (j + 1) * cap],
                        AF.Identity,
                        accum_out=st_s[:, 6 * t + 3 + j : 6 * t + 4 + j],
                    )

            out_t = statp.tile([ROWS, K], F32)
            fm = _FM(nc, statp, K)
            _emit_final_math(nc, fm, st_rxy, st_g, st_s, meta_t, out_t[:], K)
            nc.sync.dma_start(out=out_d[:], in_=out_t[:])

    return nc


# ---------------------------------------------------------------------------
# Host side
# ---------------------------------------------------------------------------
def plan_shards(num_atoms, n_classes=4, cap_round=16):
    B = num_atoms.shape[0]
    assert B % (N_CORES * ROWS) == 0
    n_classes_total = B // (N_CORES * ROWS)
    assert n_classes == n_classes_total
    order = np.argsort(num_atoms, kind="stable")
    na_sorted = num_atoms[order]
    rows_per_class = N_CORES * ROWS
    caps = []
    for k in range(n_classes):
        mx = int(na_sorted[(k + 1) * rows_per_class - 1])
        cap = ((mx + cap_round - 1) // cap_round) * cap_round
        caps.append(cap)
    return order, caps


def shard_inputs(coords_input, coords_target, num_atoms, order, caps, nmax):
    K = len(caps)
    rows_per_class = N_CORES * ROWS
    iota = np.ascontiguousarray(
        np.broadcast_to(np.arange(nmax, dtype=np.float32), (ROWS, nmax))
    )
    in_maps = []
    core_row_idx = []
    for c in range(N_CORES):
        idx = np.concatenate(
            [
                order[k * rows_per_class + c * ROWS : k * rows_per_class + (c + 1) * ROWS]
                for k in range(K)
            ]
        )
        core_row_idx.append(idx)
        nmax_l = coords_input.shape[1] // 3
        xs = np.ascontiguousarray(
            coords_input[idx].reshape(-1, nmax_l, 3).transpose(0, 2, 1).reshape(len(idx), -1)
        )
        ys = np.ascontiguousarray(
            coords_target[idx].reshape(-1, nmax_l, 3).transpose(0, 2, 1).reshape(len(idx), -1)
        )
        meta = np.ascontiguousarray(
            num_atoms[idx].astype(np.float32).reshape(K, ROWS).T
        )
        in_maps.append({"x": xs, "y": ys, "iota": iota, "meta": meta})
    return in_maps, core_row_idx


def unshard_outputs(results, core_row_idx, B):
    out = np.empty(B, dtype=np.float32)
    K = results[0]["out"].shape[1]
    for c in range(N_CORES):
        o = results[c]["out"]  # [ROWS, K]
        idx = core_row_idx[c]
        out[idx] = o.T.reshape(-1)
    return out


# ---------------------------------------------------------------------------
# Entry point: full inputs in, full output out. Shards across 8 NeuronCores.
# ---------------------------------------------------------------------------
_PROG_CACHE = {}


def _get_program(caps, nmax):
    key = (tuple(caps), nmax)
    if key not in _PROG_CACHE:
        _PROG_CACHE[key] = build_program(list(caps), nmax)
    return _PROG_CACHE[key]


def kernel(coords_input, coords_target, num_atoms):
    from concourse.bass_utils import run_bass_kernel_spmd

    x = np.ascontiguousarray(np.asarray(coords_input, dtype=np.float32))
    y = np.ascontiguousarray(np.asarray(coords_target, dtype=np.float32))
    na = np.asarray(num_atoms)
    na_i = na.astype(np.int64)
    B, ncols = x.shape
    nmax = ncols // 3
    K = B // (N_CORES * ROWS)
    assert B == N_CORES * ROWS * K, f"unsupported batch {B}"

    order, caps = plan_shards(na_i, n_classes=K)
    in_maps, core_row_idx = shard_inputs(x, y, na_i, order, caps, nmax)
    nc = _get_program(caps, nmax)
    res = run_bass_kernel_spmd(nc, in_maps, core_ids=list(range(N_CORES)))
    out = unshard_outputs(res.results, core_row_idx, B)
    return out.astype(np.float32)



# revision 3
# speedup vs baseline: 1.4445x; 1.4445x over previous
"""Bass/Trainium2 kernel for batched masked-Kabsch RMSD (nn_Coords2RMSD).

PE-centric design (SPMD across 8 cores):
  - Host sorts rows by num_atoms into 4 size classes (32 sorted tiles of
    128 rows; core c takes one tile per class). Per tile, coords are
    repacked TRANSPOSED: atoms on SBUF partitions, and for each group of
    16 rows a 112-column operand [x0 x1 x2 y0 y1 y2 1] per row (7 cols
    x 16 rows). Padding atoms are zeroed on the host; the ones column
    makes the Gram matrix carry the masked sums.
  - Per (group, chunk-of-128-atoms) ONE symmetric PE matmul
    op^T @ op accumulates into PSUM: diag 7x7 blocks per row hold all 21
    statistics (cross-covariance, |x|^2, |y|^2, sums) at once.
  - Extraction: Act copies PSUM->SBUF (bf16), 16 identity-select matmuls
    gather the diagonal blocks into a second PSUM, Act copies them into a
    row-indexed staging buffer, and 7 strided DMAs per tile-pair
    transpose [slot-comp, row] -> [row, stats].
  - Final math (centroid correction, 3x3 C^T C eigenvalues via the
    closed-form trig method, Kabsch det sign, RMSD) runs on [128, 4]
    fp32 tiles, one column per class.
"""

import numpy as np

import concourse.bass as bass
import concourse.mybir as mybir
from concourse.tile import TileContext, ScopedClock

F32 = mybir.dt.float32
BF16 = mybir.dt.bfloat16
OP = mybir.AluOpType
AF = mybir.ActivationFunctionType

N_CORES = 8
ROWS = 128          # rows per tile == final partitions
GROUPS = 8          # row-groups per tile
R = 16              # rows per group
CW = 7              # cols per row: x0 x1 x2 y0 y1 y2 1
GW = R * CW         # group operand width = 112
CHUNK = 128         # atoms per matmul pass (contraction dim)
NSTAT = CW * CW     # 49 stats per row
K = 4               # classes (tiles per core)


# ---------------------------------------------------------------------------
# TileContext tail patch: this walrus build accepts at most ONE sync-wait
# command per instruction and no sem-eq waits, so the stock drain + EVSEM
# butterfly fails codegen. Emit a ge-wait-only tail instead.
# ---------------------------------------------------------------------------
def _patched_drain_and_barrier(self, tick_clock, wait_clock):
    nc = self.nc
    dummy = nc.gpsimd.nop()
    wait_clock.add_sem_waits(dummy.ins, ScopedClock({None: tick_clock.global_clock}))
    waits = list(dummy.ins.sync_info.on_wait) if dummy.ins.sync_info else []
    if dummy.ins.sync_info:
        dummy.ins.sync_info = mybir.SyncInfo(on_wait=[], on_update=[])

    bsem = nc.alloc_semaphore(f"tail_bsem_{nc.next_id()}")
    dsem = nc.alloc_semaphore(f"tail_dsem_{nc.next_id()}")
    n_eng = 0
    for eng in nc.engines.values():
        eng.drain()
        eng.sem_inc(bsem, 1)
        n_eng += 1
    nc.gpsimd.wait_ge(bsem, n_eng)
    for w in waits:
        n = nc.gpsimd.nop()
        n.ins.sync_info = mybir.SyncInfo(on_wait=[w], on_update=[])
    nc.gpsimd.sem_inc(dsem, 1)
    for eng in nc.engines.values():
        if eng is not nc.gpsimd:
            eng.wait_ge(dsem, 1)

    popped = nc._tile_sem_poison_stack.pop()
    assert popped is self._sem_poison
    nc.clear_and_free_semaphores(list(self.sems.allocated().values()))
    nc.gpsimd.sem_clear(bsem)
    nc.gpsimd.sem_clear(dsem)


def install_tile_patch():
    TileContext._drain_and_barrier = _patched_drain_and_barrier


# ---------------------------------------------------------------------------
# BIR post-pass: split multi-wait sync infos onto NoOps (walrus accepts at
# most one sync-wait command per instruction, none on Drain).
# ---------------------------------------------------------------------------
_orig_to_json_bytes = bass.Bass.to_json_bytes


def _split_multiwait_json(self) -> bytes:
    import json

    raw = _orig_to_json_bytes(self)
    m = json.loads(raw)
    ctr = 0
    changed = False
    for f in m.get("functions", []):
        for blk in f.get("blocks", []):
            insts = blk.get("instructions", [])
            out = []
            for inst in insts:
                si = inst.get("sync_info")
                ow = (si or {}).get("on_wait") or []
                opc = str(inst.get("opcode", inst.get("type", "")))
                limit = 0 if opc == "Drain" else 1
                if len(ow) > limit:
                    keep = ow[len(ow) - limit :] if limit else []
                    moved = ow[: len(ow) - limit] if limit else ow
                    for w in moved:
                        ctr += 1
                        out.append(
                            {
                                "debug": inst.get("debug", 0),
                                "engine": inst["engine"],
                                "ins": [],
                                "name": f"WS-{ctr}-{inst['name']}",
                                "opcode": "NoOp",
                                "outs": [],
                                "sync_info": {"on_update": [], "on_wait": [w]},
                            }
                        )
                    si["on_wait"] = keep
                    changed = True
                out.append(inst)
            blk["instructions"] = out
    if not changed:
        return raw
    return json.dumps(m).encode()


bass.Bass.to_json_bytes = _split_multiwait_json


# ---------------------------------------------------------------------------
# Final math emitter on [128, K] fp32 column tiles.
# final layout: [128 rows, (t: K)(kk: 7)(cc: 7)] fp32
#   G(kk, cc) = sum_n op[n, kk] op[n, cc] per row (kk,cc in 0..5 = comps,
#   6 = ones => sums). Columns for class t at offset t*49.
# ---------------------------------------------------------------------------
class _FM:
    def __init__(self, nc, pool, Kn):
        self.nc = nc
        self.pool = pool
        self.K = Kn
        self.n = 0
        self._consts = {}

    def const_col(self, val):
        val = float(val)
        if val in self._consts:
            return self._consts[val]
        i = len(self._consts)
        t = self.pool.tile([ROWS, 1], F32, tag=f"fmc{i}", name=f"fmc{i}")
        self.nc.vector.memset(t[:], val)
        self._consts[val] = t[:]
        return t[:]

    def t(self, w=None):
        self.n += 1
        return self.pool.tile(
            [ROWS, w or self.K], F32, tag=f"fm{self.n}", name=f"fm{self.n}"
        )

    def tt(self, a, b, op):
        o = self.t()
        self.nc.vector.tensor_tensor(o[:], a, b, op)
        return o[:]

    def mul(self, a, b):
        return self.tt(a, b, OP.mult)

    def add(self, a, b):
        return self.tt(a, b, OP.add)

    def sub(self, a, b):
        return self.tt(a, b, OP.subtract)

    def ts(self, a, s, op):
        o = self.t()
        self.nc.vector.tensor_scalar(o[:], a, float(s), None, op)
        return o[:]

    def ts2(self, a, s1, s2, op0, op1):
        o = self.t()
        self.nc.vector.tensor_scalar(o[:], a, float(s1), float(s2), op0, op1)
        return o[:]

    def stt(self, a, s, b, op0, op1):
        """(a op0 s) op1 b"""
        o = self.t()
        self.nc.vector.scalar_tensor_tensor(o[:], a, float(s), b, op0, op1)
        return o[:]

    def act(self, a, func, bias=0.0, scale=1.0):
        o = self.t()
        if isinstance(bias, float) and bias not in (0.0, 1.0) and func != AF.Copy:
            bias = self.const_col(bias)
        self.nc.scalar.activation(o[:], a, func, bias=bias, scale=scale)
        return o[:]

    def recip(self, a):
        o = self.t()
        self.nc.vector.reciprocal(o[:], a)
        return o[:]


def _emit_final_math(nc, fm, final_t, meta_t, out_ap, Kn):
    fv = final_t[:].rearrange("p (t k c) -> p t k c", t=Kn, k=CW)

    def G(kk, cc):  # [128, K]
        return fv[:, :, kk, cc]

    n_ap = meta_t[:]
    rn = fm.recip(n_ap)

    def Sx(i):
        return G(6, i)

    def Sy(j):
        return G(6, 3 + j)

    # sxn_i = Sx_i * rn  (3 ops)
    sxn = [fm.mul(Sx(i), rn) for i in range(3)]
    # C_ij = G(i, 3+j) - sxn_i * Sy_j  (18 ops)
    C = [[None] * 3 for _ in range(3)]
    for i in range(3):
        for j in range(3):
            C[i][j] = fm.sub(G(i, 3 + j), fm.mul(sxn[i], Sy(j)))

    # gx = (G00+G11+G22) - (Sx.Sx) rn ; gy analog
    qx = fm.add(fm.add(G(0, 0), G(1, 1)), G(2, 2))
    qy = fm.add(fm.add(G(3, 3), G(4, 4)), G(5, 5))
    sx2 = fm.add(fm.add(fm.mul(sxn[0], Sx(0)), fm.mul(sxn[1], Sx(1))), fm.mul(sxn[2], Sx(2)))
    syn = [fm.mul(Sy(j), rn) for j in range(3)]
    sy2 = fm.add(fm.add(fm.mul(syn[0], Sy(0)), fm.mul(syn[1], Sy(1))), fm.mul(syn[2], Sy(2)))
    gx = fm.sub(qx, sx2)
    gy = fm.sub(qy, sy2)

    # M = C^T C (symmetric; M[a][b] = sum_i C[i][a] C[i][b])
    kk = {}
    for a in range(3):
        for b in range(a, 3):
            s = fm.mul(C[0][a], C[0][b])
            s = fm.add(s, fm.mul(C[1][a], C[1][b]))
            s = fm.add(s, fm.mul(C[2][a], C[2][b]))
            kk[(a, b)] = s

    # det(C)
    m0 = fm.sub(fm.mul(C[1][1], C[2][2]), fm.mul(C[1][2], C[2][1]))
    m1 = fm.sub(fm.mul(C[1][0], C[2][2]), fm.mul(C[1][2], C[2][0]))
    m2 = fm.sub(fm.mul(C[1][0], C[2][1]), fm.mul(C[1][1], C[2][0]))
    detC = fm.add(fm.sub(fm.mul(C[0][0], m0), fm.mul(C[0][1], m1)), fm.mul(C[0][2], m2))

    # q = tr(M)/3 ; p2 = sum (M_aa - q)^2 + 2 (M01^2 + M02^2 + M12^2)
    q = fm.ts(fm.add(fm.add(kk[(0, 0)], kk[(1, 1)]), kk[(2, 2)]), 1.0 / 3.0, OP.mult)
    kd = [fm.sub(kk[(a, a)], q) for a in range(3)]
    p2 = fm.add(fm.add(fm.mul(kd[0], kd[0]), fm.mul(kd[1], kd[1])), fm.mul(kd[2], kd[2]))
    xsq = fm.add(
        fm.add(fm.mul(kk[(0, 1)], kk[(0, 1)]), fm.mul(kk[(0, 2)], kk[(0, 2)])),
        fm.mul(kk[(1, 2)], kk[(1, 2)]),
    )
    p2 = fm.stt(xsq, 2.0, p2, OP.mult, OP.add)  # p2 + 2*xsq
    p2c = fm.ts(fm.ts(p2, 1.0 / 6.0, OP.mult), 1e-30, OP.max)
    p = fm.act(p2c, AF.Sqrt)

    # det(M - qI)
    k01, k02, k12 = kk[(0, 1)], kk[(0, 2)], kk[(1, 2)]
    d0 = fm.mul(kd[0], fm.sub(fm.mul(kd[1], kd[2]), fm.mul(k12, k12)))
    d1 = fm.mul(k01, fm.sub(fm.mul(k01, kd[2]), fm.mul(k12, k02)))
    d2 = fm.mul(k02, fm.sub(fm.mul(k01, k12), fm.mul(kd[1], k02)))
    detKq = fm.add(fm.sub(d0, d1), d2)

    # r = 0.5 det(M-qI) / p^3, clamped to [-1, 1]
    rp = fm.recip(p)
    r = fm.mul(fm.mul(fm.ts(detKq, 0.5, OP.mult), rp), fm.mul(rp, rp))
    r = fm.ts(fm.ts(r, 1.0, OP.min), -1.0, OP.max)

    # acos via |r| fold; phi = acos(r)/3
    rabs = fm.stt(r, -1.0, r, OP.mult, OP.max)  # |r|
    onemr = fm.act(rabs, AF.Identity, bias=1.0, scale=-1.0)  # 1 - |r|
    onepr = fm.ts(rabs, 1.0, OP.add)
    u = fm.mul(onemr, fm.recip(onepr))
    su = fm.act(u, AF.Sqrt)
    at = fm.act(su, AF.Arctan)
    A = fm.ts(at, 2.0, OP.mult)
    rneg = fm.ts(r, 0.0, OP.is_lt)
    corr = fm.ts2(A, -2.0, float(np.pi), OP.mult, OP.add)  # pi - 2A
    acr = fm.add(A, fm.mul(rneg, corr))
    c1 = fm.act(acr, AF.Sin, bias=float(np.pi / 2), scale=1.0 / 3.0)
    c3m = fm.act(acr, AF.Sin, bias=float(5 * np.pi / 6), scale=-1.0 / 3.0)

    p2x = fm.ts(p, 2.0, OP.mult)
    l1 = fm.add(q, fm.mul(p2x, c1))
    l3 = fm.sub(q, fm.mul(p2x, c3m))
    l2 = fm.sub(fm.stt(q, 3.0, l1, OP.mult, OP.subtract), l3)  # 3q - l1 - l3

    s1 = fm.act(fm.ts(l1, 0.0, OP.max), AF.Sqrt)
    s2 = fm.act(fm.ts(l2, 0.0, OP.max), AF.Sqrt)
    s3 = fm.act(fm.ts(l3, 0.0, OP.max), AF.Sqrt)

    # d = +1 if detC >= 0 else -1
    neg = fm.ts(detC, 0.0, OP.is_lt)
    d = fm.act(neg, AF.Identity, bias=1.0, scale=-2.0)

    tr = fm.add(fm.add(s1, s2), fm.mul(d, s3))

    # msd = (gx + gy - 2 tr) rn ; rmsd = sqrt(max(msd, 0))
    diff = fm.stt(tr, -2.0, fm.add(gx, gy), OP.mult, OP.add)
    msd = fm.mul(diff, rn)
    rmsd = fm.act(fm.ts(msd, 0.0, OP.max), AF.Sqrt)
    nc.vector.tensor_copy(out_ap, rmsd)


# ---------------------------------------------------------------------------
# Program builder
# ---------------------------------------------------------------------------
def build_program(chunks, cfg=None):
    """chunks: per-class chunk counts (len K). Returns nc."""
    cfg = cfg or {}
    do_mm = cfg.get("mm", True)
    do_extract = cfg.get("extract", True)
    do_math = cfg.get("math", True)
    Kn = len(chunks)
    install_tile_patch()
    nc = bass.Bass()
    op_d = [
        nc.dram_tensor(f"op{t}", [ROWS, chunks[t] * GROUPS * GW], BF16,
                       kind="ExternalInput")
        for t in range(Kn)
    ]
    sel_d = nc.dram_tensor("sel", [GW, R * CW], BF16, kind="ExternalInput")
    meta_d = nc.dram_tensor("meta", [ROWS, Kn], F32, kind="ExternalInput")
    out_d = nc.dram_tensor("out", [ROWS, Kn], F32, kind="ExternalOutput")

    with TileContext(nc) as tc:
        with (
            tc.tile_pool(name="const", bufs=1) as constp,
            tc.tile_pool(name="ops", bufs=1) as opp,
            tc.tile_pool(name="gsb", bufs=2) as gsbp,
            tc.tile_pool(name="ext", bufs=1) as extp,
            tc.tile_pool(name="fmp", bufs=1) as fmp,
            tc.tile_pool(name="psA", bufs=2, space="PSUM") as psA,
            tc.tile_pool(name="psB", bufs=2, space="PSUM") as psB,
        ):
            sel_t = constp.tile([GW, R * CW], BF16)
            nc.sync.dma_start(out=sel_t[:], in_=sel_d[:])
            meta_t = constp.tile([ROWS, Kn], F32)
            nc.sync.dma_start(out=meta_t[:], in_=meta_d[:])

            # staging for rows: ext [7, (r 16)(g 8)(t 2)(c 7)] per pair
            exts = [
                extp.tile([CW, R * GROUPS * 2 * CW], F32, name=f"extp{p}")
                for p in range(Kn // 2)
            ]
            final_t = fmp.tile([ROWS, Kn * NSTAT], F32)

            op_t = []
            for t in range(Kn):
                op = opp.tile([ROWS, chunks[t] * GROUPS * GW], BF16, name=f"op{t}")
                nc.sync.dma_start(out=op[:], in_=op_d[t][:])
                op_t.append(op)

            for t in range(Kn if do_mm else 0):
                Ct = chunks[t]
                op = op_t[t]
                gram = psA.tile([128, 1024], F32, tag="gram")  # 2 banks
                gv = gram[:].rearrange("p (g w) -> p g w", g=GROUPS)
                for g in range(GROUPS):
                    for c in range(Ct):
                        sl = op[:, (c * GROUPS + g) * GW : (c * GROUPS + g + 1) * GW]
                        nc.tensor.matmul(
                            gv[0:GW, g, 0:GW], sl, sl,
                            start=(c == 0), stop=(c == Ct - 1),
                            skip_group_check=True,
                        )
                # Act copy1: full gram -> SBUF bf16
                if not do_extract:
                    continue
                gram_sb = gsbp.tile([GW, GROUPS * GW], BF16, tag="gramsb")
                gsv = gram_sb[:].rearrange("p (g w) -> p g w", g=GROUPS)
                nc.scalar.activation(gsv[:, :, :], gv[0:GW, :, 0:GW], AF.Copy)
                # 16 select matmuls: slot r -> psum2 [7, r*64 + (g 8)(c 7)]
                ps2 = psB.tile([128, 1024], F32, tag="ps2")  # 2 banks
                p2v = ps2[:].rearrange("p (r w) -> p r w", r=R)
                for r in range(R):
                    rhs = gsv[:, :, CW * r : CW * r + CW]  # [112, 8, 7]
                    lhsT = sel_t[:, CW * r : CW * r + CW]  # [112, 7] identity slice
                    nc.tensor.matmul(
                        p2v[0:CW, r, 0 : GROUPS * CW], lhsT, rhs,
                        start=True, stop=True, skip_group_check=True,
                    )
                # Act copy2: psum2 -> ext[:, (r)(g)(tp)(c)]
                pair, tp = divmod(t, 2)
                ev = exts[pair][:].rearrange(
                    "p (r g t c) -> p r g t c", r=R, g=GROUPS, t=2
                )
                nc.scalar.activation(
                    ev[:, :, :, tp, :],
                    p2v[0:CW, :, 0 : GROUPS * CW].rearrange(
                        "p r (g c) -> p r g c", g=GROUPS
                    ),
                    AF.Copy,
                )
                # after both tiles of a pair: 7 transpose DMAs -> final
                if tp == 1:
                    fvv = final_t[:].rearrange(
                        "p (t k c) -> p t k c", t=Kn, k=CW
                    )
                    for kkc in range(CW):
                        nc.scalar.dma_start(
                            out=fvv[:, 2 * pair : 2 * pair + 2, kkc, :],
                            in_=ev[kkc : kkc + 1, :, :, :, :],
                        )

            out_t = fmp.tile([ROWS, Kn], F32)
            if do_math:
                fm = _FM(nc, fmp, Kn)
                _emit_final_math(nc, fm, final_t, meta_t, out_t[:], Kn)
            else:
                nc.vector.memset(out_t[:], 0.0)
            nc.sync.dma_start(out=out_d[:], in_=out_t[:])

    return nc


# ---------------------------------------------------------------------------
# Host side
# ---------------------------------------------------------------------------
def plan_shards(num_atoms, n_classes=K):
    B = num_atoms.shape[0]
    assert B % (N_CORES * ROWS) == 0
    assert n_classes == B // (N_CORES * ROWS)
    order = np.argsort(num_atoms, kind="stable")
    na_sorted = num_atoms[order]
    rows_per_class = N_CORES * ROWS
    chunks = []
    for k in range(n_classes):
        mx = int(na_sorted[(k + 1) * rows_per_class - 1])
        chunks.append((mx + CHUNK - 1) // CHUNK)
    return order, chunks


def _pack_tile(x, y, na, Ct):
    """x, y: [128, nmax, 3] f32 (row-major positions), na: [128] int.
    Returns op [128, Ct, GROUPS, GW] f32 with atoms on dim 0 (partitions)."""
    nmax = x.shape[1]
    cap = Ct * CHUNK
    # data [b, n, 7]
    d = np.zeros((ROWS, cap, CW), np.float32)
    ncl = min(cap, nmax)
    d[:, :ncl, 0:3] = x[:, :ncl, :]
    d[:, :ncl, 3:6] = y[:, :ncl, :]
    mask = (np.arange(cap)[None, :] < na[:, None]).astype(np.float32)
    d[:, :, 0:6] *= mask[:, :, None]
    d[:, :, 6] = 1.0
    # op[p, c, g, 7r+k] = d[8r+g, c*128+p, k]
    d = d.reshape(ROWS, Ct, CHUNK, CW)            # [b, c, p, k]
    d = d.transpose(2, 1, 0, 3)                   # [p, c, b, k]
    d = d.reshape(CHUNK, Ct, R, GROUPS, CW)       # [p, c, r, g, k]  (b = 8r+g)
    d = d.transpose(0, 1, 3, 2, 4)                # [p, c, g, r, k]
    return np.ascontiguousarray(d.reshape(CHUNK, Ct, GROUPS, GW))


def shard_inputs(coords_input, coords_target, num_atoms, order, chunks):
    import ml_dtypes

    B, ncols = coords_input.shape
    nmax = ncols // 3
    Kn = len(chunks)
    rows_per_class = N_CORES * ROWS
    sel = np.zeros((GW, R * CW), np.float32)
    for j in range(R * CW):
        sel[j, j] = 1.0
    sel = sel.astype(ml_dtypes.bfloat16)
    in_maps = []
    core_row_idx = []
    for c in range(N_CORES):
        m = {"sel": sel}
        idx_all = []
        meta = np.zeros((ROWS, Kn), np.float32)
        for t in range(Kn):
            idx = order[t * rows_per_class + c * ROWS : t * rows_per_class + (c + 1) * ROWS]
            idx_all.append(idx)
            na = num_atoms[idx]
            meta[:, t] = na.astype(np.float32)
            x = coords_input[idx].reshape(ROWS, nmax, 3)
            y = coords_target[idx].reshape(ROWS, nmax, 3)
            op = _pack_tile(x, y, na, chunks[t])
            m[f"op{t}"] = np.ascontiguousarray(
                op.reshape(CHUNK, -1)
            ).astype(ml_dtypes.bfloat16)
        m["meta"] = meta
        in_maps.append(m)
        core_row_idx.append(np.concatenate(idx_all))
    return in_maps, core_row_idx


def unshard_outputs(results, core_row_idx, B):
    out = np.empty(B, dtype=np.float32)
    for c in range(N_CORES):
        o = results[c]["out"]  # [ROWS, K]
        out[core_row_idx[c]] = o.T.reshape(-1)
    return out


# ---------------------------------------------------------------------------
# Entry point
# ---------------------------------------------------------------------------
_PROG_CACHE = {}


def _get_program(chunks):
    key = tuple(chunks)
    if key not in _PROG_CACHE:
        _PROG_CACHE[key] = build_program(list(chunks))
    return _PROG_CACHE[key]


def kernel(coords_input, coords_target, num_atoms):
    from concourse.bass_utils import run_bass_kernel_spmd

    x = np.ascontiguousarray(np.asarray(coords_input, dtype=np.float32))
    y = np.ascontiguousarray(np.asarray(coords_target, dtype=np.float32))
    na = np.asarray(num_atoms).astype(np.int64)
    B, ncols = x.shape
    Kn = B // (N_CORES * ROWS)
    assert B == N_CORES * ROWS * Kn, f"unsupported batch {B}"

    order, chunks = plan_shards(na, n_classes=Kn)
    in_maps, core_row_idx = shard_inputs(x, y, na, order, chunks)
    nc = _get_program(chunks)
    res = run_bass_kernel_spmd(nc, in_maps, core_ids=list(range(N_CORES)))
    out = unshard_outputs(res.results, core_row_idx, B)
    return out.astype(np.float32)


# revision 16
# speedup vs baseline: 2.4639x; 1.7057x over previous
"""Bass/Trainium2 kernel for batched masked-Kabsch RMSD (nn_Coords2RMSD).

PE-centric design (SPMD across 8 cores):
  - Host sorts rows by num_atoms into 4 size classes (32 sorted tiles of
    128 rows; core c takes one tile per class). Per tile, coords are
    repacked TRANSPOSED: atoms on SBUF partitions, and for each group of
    16 rows a 112-column operand [x0 x1 x2 y0 y1 y2 1] per row (7 cols
    x 16 rows). Padding atoms are zeroed on the host; the ones column
    makes the Gram matrix carry the masked sums.
  - Per (group, chunk-of-128-atoms) ONE symmetric PE matmul
    op^T @ op accumulates into PSUM: diag 7x7 blocks per row hold all 21
    statistics (cross-covariance, |x|^2, |y|^2, sums) at once.
  - Extraction: Act copies PSUM->SBUF (bf16), 16 identity-select matmuls
    gather the diagonal blocks into a second PSUM, Act copies them into a
    row-indexed staging buffer, and 7 strided DMAs per tile-pair
    transpose [slot-comp, row] -> [row, stats].
  - Final math (centroid correction, 3x3 C^T C eigenvalues via the
    closed-form trig method, Kabsch det sign, RMSD) runs on [128, 4]
    fp32 tiles, one column per class.
"""

import numpy as np

import concourse.bass as bass
import concourse.mybir as mybir
from concourse.tile import TileContext, ScopedClock

F32 = mybir.dt.float32
BF16 = mybir.dt.bfloat16
FP8 = mybir.dt.float8e4
OP_DT = FP8  # gram operand dtype (host-cast)
OP = mybir.AluOpType
AF = mybir.ActivationFunctionType

N_CORES = 8
ROWS = 128          # rows per tile == final partitions
GROUPS = 8          # row-groups per tile
R = 16              # rows per group
CW = 7              # cols per row: x0 x1 x2 y0 y1 y2 1
GW = R * CW         # group operand width = 112
CHUNK = 128         # atoms per matmul pass (contraction dim)
NSTAT = CW * CW     # 49 stats per row
K = 4               # classes (tiles per core)


# ---------------------------------------------------------------------------
# TileContext tail patch: this walrus build accepts at most ONE sync-wait
# command per instruction and no sem-eq waits, so the stock drain + EVSEM
# butterfly fails codegen. Emit a ge-wait-only tail instead.
# ---------------------------------------------------------------------------
def _patched_drain_and_barrier(self, tick_clock, wait_clock):
    nc = self.nc
    dummy = nc.gpsimd.nop()
    wait_clock.add_sem_waits(dummy.ins, ScopedClock({None: tick_clock.global_clock}))
    waits = list(dummy.ins.sync_info.on_wait) if dummy.ins.sync_info else []
    if dummy.ins.sync_info:
        dummy.ins.sync_info = mybir.SyncInfo(on_wait=[], on_update=[])

    bsem = nc.alloc_semaphore(f"tail_bsem_{nc.next_id()}")
    dsem = nc.alloc_semaphore(f"tail_dsem_{nc.next_id()}")
    n_eng = 0
    for eng in nc.engines.values():
        eng.drain()
        eng.sem_inc(bsem, 1)
        n_eng += 1
    nc.gpsimd.wait_ge(bsem, n_eng)
    for w in waits:
        n = nc.gpsimd.nop()
        n.ins.sync_info = mybir.SyncInfo(on_wait=[w], on_update=[])
    nc.gpsimd.sem_inc(dsem, 1)
    for eng in nc.engines.values():
        if eng is not nc.gpsimd:
            eng.wait_ge(dsem, 1)

    popped = nc._tile_sem_poison_stack.pop()
    assert popped is self._sem_poison
    nc.clear_and_free_semaphores(list(self.sems.allocated().values()))
    nc.gpsimd.sem_clear(bsem)
    nc.gpsimd.sem_clear(dsem)


def install_tile_patch():
    TileContext._drain_and_barrier = _patched_drain_and_barrier


# ---------------------------------------------------------------------------
# BIR post-pass: split multi-wait sync infos onto NoOps (walrus accepts at
# most one sync-wait command per instruction, none on Drain).
# ---------------------------------------------------------------------------
_orig_to_json_bytes = bass.Bass.to_json_bytes


def _split_multiwait_json(self) -> bytes:
    import json

    raw = _orig_to_json_bytes(self)
    m = json.loads(raw)
    ctr = 0
    changed = False
    for f in m.get("functions", []):
        for blk in f.get("blocks", []):
            insts = blk.get("instructions", [])
            out = []
            for inst in insts:
                si = inst.get("sync_info")
                ow = (si or {}).get("on_wait") or []
                opc = str(inst.get("opcode", inst.get("type", "")))
                limit = 0 if opc == "Drain" else 1
                if len(ow) > limit:
                    keep = ow[len(ow) - limit :] if limit else []
                    moved = ow[: len(ow) - limit] if limit else ow
                    for w in moved:
                        ctr += 1
                        out.append(
                            {
                                "debug": inst.get("debug", 0),
                                "engine": inst["engine"],
                                "ins": [],
                                "name": f"WS-{ctr}-{inst['name']}",
                                "opcode": "NoOp",
                                "outs": [],
                                "sync_info": {"on_update": [], "on_wait": [w]},
                            }
                        )
                    si["on_wait"] = keep
                    changed = True
                out.append(inst)
            blk["instructions"] = out
    if not changed:
        return raw
    return json.dumps(m).encode()


bass.Bass.to_json_bytes = _split_multiwait_json


# ---------------------------------------------------------------------------
# Final math emitter on [128, K] fp32 column tiles.
# final layout: [128 rows, (t: K)(kk: 7)(cc: 7)] fp32
#   G(kk, cc) = sum_n op[n, kk] op[n, cc] per row (kk,cc in 0..5 = comps,
#   6 = ones => sums). Columns for class t at offset t*49.
# ---------------------------------------------------------------------------
class _FM:
    def __init__(self, nc, pool, Kn, prefix=""):
        self.nc = nc
        self.pool = pool
        self.K = Kn
        self.n = 0
        self.prefix = prefix
        self._consts = {}

    def const_col(self, val):
        val = float(val)
        if val in self._consts:
            return self._consts[val]
        i = len(self._consts)
        t = self.pool.tile([ROWS, 1], F32, tag=f"fmc{i}", name=f"fmc{i}")
        self.nc.vector.memset(t[:], val)
        self._consts[val] = t[:]
        return t[:]

    def t(self, w=None):
        self.n += 1
        nm = f"fm{self.prefix}{self.n}"
        return self.pool.tile([ROWS, w or self.K], F32, tag=nm, name=nm)

    def tt(self, a, b, op):
        o = self.t()
        self.nc.vector.tensor_tensor(o[:], a, b, op)
        return o[:]

    def mul(self, a, b):
        return self.tt(a, b, OP.mult)

    def add(self, a, b):
        return self.tt(a, b, OP.add)

    def sub(self, a, b):
        return self.tt(a, b, OP.subtract)

    def ts(self, a, s, op):
        o = self.t()
        self.nc.vector.tensor_scalar(o[:], a, float(s), None, op)
        return o[:]

    def ts2(self, a, s1, s2, op0, op1):
        o = self.t()
        self.nc.vector.tensor_scalar(o[:], a, float(s1), float(s2), op0, op1)
        return o[:]

    def stt(self, a, s, b, op0, op1):
        """(a op0 s) op1 b"""
        o = self.t()
        self.nc.vector.scalar_tensor_tensor(o[:], a, float(s), b, op0, op1)
        return o[:]

    def act(self, a, func, bias=0.0, scale=1.0):
        o = self.t()
        if isinstance(bias, float) and bias not in (0.0, 1.0) and func != AF.Copy:
            bias = self.const_col(bias)
        self.nc.scalar.activation(o[:], a, func, bias=bias, scale=scale)
        return o[:]

    def recip(self, a):
        o = self.t()
        self.nc.vector.reciprocal(o[:], a)
        return o[:]


def _emit_math_pair(nc, fm, final_t, meta_ap, out_ap, Kn, t0, Kp):
    """Wide-op final math for classes [t0, t0+Kp)."""
    fv = final_t[:].rearrange("p (t k c) -> p t k c", t=Kn, k=CW)[
        :, t0 : t0 + Kp, :, :
    ]
    fvf = final_t[:].rearrange("p (t c) -> p t c", t=Kn)[
        :, t0 : t0 + Kp, :
    ]

    def W(w):  # fresh wide tile
        return fm.t(w)

    rn = fm.recip(meta_ap)  # [128, Kp]
    rn_b3 = rn[:, :, None].broadcast_to([ROWS, Kp, 3])

    P = fv[:, :, 0:3, 3:6]          # [128, 2, 3, 3]
    Sx = fv[:, :, 6, 0:3]           # [128, 2, 3]
    Sy = fv[:, :, 6, 3:6]

    sxn_t = W(Kp * 3)
    sxn = sxn_t[:].rearrange("p (t c) -> p t c", t=Kp)
    nc.vector.tensor_tensor(sxn, Sx, rn_b3, OP.mult)

    t1_t = W(Kp * 9)
    t1 = t1_t[:].rearrange("p (t i j) -> p t i j", t=Kp, i=3)
    nc.vector.tensor_tensor(
        t1, sxn[:, :, :, None].broadcast_to([ROWS, Kp, 3, 3]),
        Sy[:, :, None, :].broadcast_to([ROWS, Kp, 3, 3]), OP.mult)
    C_t = W(Kp * 9)
    C = C_t[:].rearrange("p (t i j) -> p t i j", t=Kp, i=3)
    nc.vector.tensor_tensor(C, P, t1, OP.subtract)

    def Cij(i, j):
        return C[:, :, i, j]

    # M = C^T C via 3 outer products
    M_t = W(Kp * 9)
    M = M_t[:].rearrange("p (t a b) -> p t a b", t=Kp, a=3)
    tmp_t = W(Kp * 9)
    tmp = tmp_t[:].rearrange("p (t a b) -> p t a b", t=Kp, a=3)
    for i in range(3):
        Ci = C[:, :, i, :]
        dst = M if i == 0 else tmp
        nc.vector.tensor_tensor(
            dst, Ci[:, :, :, None].broadcast_to([ROWS, Kp, 3, 3]),
            Ci[:, :, None, :].broadcast_to([ROWS, Kp, 3, 3]), OP.mult)
        if i > 0:
            nc.vector.tensor_tensor(M, M, tmp, OP.add)

    Mf = M_t[:].rearrange("p (t ab) -> p t ab", t=Kp)
    Mdiag = Mf[:, :, 0:9:4]  # [128, 2, 3]

    # q = trM/3
    q = fm.add(Mdiag[:, :, 0], Mdiag[:, :, 1])
    q = fm.stt(Mdiag[:, :, 2], 1.0, q, OP.mult, OP.add)
    q = fm.ts(q, 1.0 / 3.0, OP.mult)

    # trM2 = sum M*M ; p2 = trM2 - 3 q^2
    MM_t = W(Kp * 9)
    nc.vector.tensor_tensor(MM_t[:], M_t[:], M_t[:], OP.mult)
    trM2 = fm.t()
    nc.vector.tensor_reduce(
        trM2[:], MM_t[:].rearrange("p (t ab) -> p t ab", t=Kp),
        mybir.AxisListType.X, OP.add)
    qq = fm.mul(q, q)
    p2 = fm.stt(qq, -3.0, trM2[:], OP.mult, OP.add)
    p2c = fm.ts2(p2, 1.0 / 6.0, 1e-30, OP.mult, OP.max)
    p = fm.act(p2c, AF.Sqrt)

    # --- detC, detC^2, sign (DVE; pool per-op overhead hurts the chain) ---
    def gtt(a, b, op):
        o = fm.t()
        nc.vector.tensor_tensor(o[:], a, b, op)
        return o[:]

    gm0 = gtt(Cij(1, 1), Cij(2, 2), OP.mult)
    gm0b = gtt(Cij(1, 2), Cij(2, 1), OP.mult)
    gm0 = gtt(gm0, gm0b, OP.subtract)
    gm1 = gtt(Cij(1, 0), Cij(2, 2), OP.mult)
    gm1b = gtt(Cij(1, 2), Cij(2, 0), OP.mult)
    gm1 = gtt(gm1, gm1b, OP.subtract)
    gm2 = gtt(Cij(1, 0), Cij(2, 1), OP.mult)
    gm2b = gtt(Cij(1, 1), Cij(2, 0), OP.mult)
    gm2 = gtt(gm2, gm2b, OP.subtract)
    d0 = gtt(Cij(0, 0), gm0, OP.mult)
    d1 = gtt(Cij(0, 1), gm1, OP.mult)
    d2 = gtt(Cij(0, 2), gm2, OP.mult)
    detC = gtt(gtt(d0, d1, OP.subtract), d2, OP.add)
    detC2 = gtt(detC, detC, OP.mult)
    dneg = fm.t()
    nc.vector.tensor_scalar(dneg[:], detC, 0.0, None, OP.is_lt)
    dsign = fm.t()
    nc.vector.tensor_scalar(dsign[:], dneg[:], -2.0, 1.0, OP.mult, OP.add)

    # detKq = det(M - qI) = -2.5 q^3 + 0.5 q trM2 + detC^2
    q3 = fm.mul(qq, q)
    a_ = fm.mul(q, trM2[:])
    t_ = fm.stt(a_, 0.5, detC2, OP.mult, OP.add)
    detKq = fm.stt(q3, -2.5, t_, OP.mult, OP.add)

    # r = 0.5 detKq / p^3 clamped
    rp = fm.recip(p)
    rp2 = fm.mul(rp, rp)
    r = fm.mul(fm.mul(fm.ts(detKq, 0.5, OP.mult), rp), rp2)
    r = fm.ts(fm.ts(r, 1.0, OP.min), -1.0, OP.max)

    # Newton on 4c^3-3c=r for c1 (cos(phi)) and c3 (cos(phi+2pi/3)), packed
    A2, A1, A0 = -0.07910172, 0.19285723, 0.87011722
    rr = fm.mul(r, r)
    cpack_t = W(2 * Kp)
    cpack = cpack_t[:].rearrange("p (s t) -> p s t", s=2)
    ta = fm.ts2(r, A1, A0, OP.mult, OP.add)
    nc.vector.scalar_tensor_tensor(cpack[:, 0, :], rr, A2, ta, OP.mult, OP.add)
    tb = fm.ts2(r, A1, -A0, OP.mult, OP.add)
    nc.vector.scalar_tensor_tensor(cpack[:, 1, :], rr, -A2, tb, OP.mult, OP.add)
    r_b = r[:, None, :].broadcast_to([ROWS, 2, Kp])
    for _ in range(3):
        c2 = fm.t(2 * Kp)
        nc.vector.tensor_tensor(c2[:], cpack_t[:], cpack_t[:], OP.mult)
        c3 = fm.t(2 * Kp)
        nc.vector.tensor_tensor(c3[:], c2[:], cpack_t[:], OP.mult)
        num = fm.t(2 * Kp)
        nc.vector.scalar_tensor_tensor(
            num[:].rearrange("p (s t) -> p s t", s=2),
            c3[:].rearrange("p (s t) -> p s t", s=2), 8.0, r_b,
            OP.mult, OP.add)
        den = fm.t(2 * Kp)
        nc.vector.tensor_scalar(den[:], c2[:], 12.0, -3.0, OP.mult, OP.add)
        rec = fm.t(2 * Kp)
        nc.vector.reciprocal(rec[:], den[:])
        nc.vector.tensor_tensor(cpack_t[:], num[:], rec[:], OP.mult)

    # lambdas: l1 = q + 2p c1 ; l3 = q + 2p c3 ; l2 = 3q - l1 - l3
    p2x = fm.ts(p, 2.0, OP.mult)
    lpack_t = W(3 * Kp)
    lpack = lpack_t[:].rearrange("p (s t) -> p s t", s=3)
    t_l1 = fm.mul(p2x, cpack[:, 0, :])
    nc.vector.tensor_tensor(lpack[:, 0, :], q, t_l1, OP.add)
    t_l3 = fm.mul(p2x, cpack[:, 1, :])
    nc.vector.tensor_tensor(lpack[:, 1, :], q, t_l3, OP.add)
    t_l2 = fm.stt(q, 3.0, lpack[:, 0, :], OP.mult, OP.subtract)
    nc.vector.tensor_tensor(lpack[:, 2, :], t_l2, lpack[:, 1, :], OP.subtract)
    lmax = fm.t(3 * Kp)
    nc.vector.tensor_scalar(lmax[:], lpack_t[:], 0.0, None, OP.max)
    spack_t = fm.t(3 * Kp)
    nc.scalar.activation(spack_t[:], lmax[:], AF.Sqrt)
    spack = spack_t[:].rearrange("p (s t) -> p s t", s=3)

    tr = fm.add(spack[:, 0, :], spack[:, 2, :])
    tr = fm.add(tr, fm.mul(dsign[:], spack[:, 1, :]))

    # gx + gy
    Qx = fm.t()
    nc.vector.tensor_reduce(Qx[:], fvf[:, :, 0:17:8], mybir.AxisListType.X, OP.add)
    Qy = fm.t()
    nc.vector.tensor_reduce(Qy[:], fvf[:, :, 24:41:8], mybir.AxisListType.X, OP.add)
    sxx_t = W(Kp * 3)
    nc.vector.tensor_tensor(
        sxx_t[:].rearrange("p (t c) -> p t c", t=Kp), sxn, Sx, OP.mult)
    sx2 = fm.t()
    nc.vector.tensor_reduce(
        sx2[:], sxx_t[:].rearrange("p (t c) -> p t c", t=Kp),
        mybir.AxisListType.X, OP.add)
    syn_t = W(Kp * 3)
    syn = syn_t[:].rearrange("p (t c) -> p t c", t=Kp)
    nc.vector.tensor_tensor(syn, Sy, rn_b3, OP.mult)
    syy_t = W(Kp * 3)
    nc.vector.tensor_tensor(
        syy_t[:].rearrange("p (t c) -> p t c", t=Kp), syn, Sy, OP.mult)
    sy2 = fm.t()
    nc.vector.tensor_reduce(
        sy2[:], syy_t[:].rearrange("p (t c) -> p t c", t=Kp),
        mybir.AxisListType.X, OP.add)
    g = fm.sub(fm.add(Qx[:], Qy[:]), fm.add(sx2[:], sy2[:]))

    diff = fm.stt(tr, -2.0, g, OP.mult, OP.add)
    msd = fm.mul(diff, rn)
    rmsd = fm.act(fm.ts(msd, 0.0, OP.max), AF.Sqrt)
    nc.vector.tensor_copy(out_ap, rmsd)


# ---------------------------------------------------------------------------
# Program builder
# ---------------------------------------------------------------------------
def build_program(chunks, cfg=None):
    """chunks: per-class chunk counts (len K). Returns nc."""
    cfg = cfg or {}
    do_mm = cfg.get("mm", True)
    do_extract = cfg.get("extract", True)
    do_math = cfg.get("math", True)
    Kn = len(chunks)
    install_tile_patch()
    nc = bass.Bass()
    op_dt = FP8 if cfg.get("fp8", True) else BF16
    op_d = [
        nc.dram_tensor(f"op{t}", [ROWS, chunks[t] * GROUPS * GW], op_dt,
                       kind="ExternalInput")
        for t in range(Kn)
    ]
    sel_d = nc.dram_tensor("sel", [GW, R * CW], BF16, kind="ExternalInput")
    meta_d = nc.dram_tensor("meta", [ROWS, Kn], F32, kind="ExternalInput")
    out_d = nc.dram_tensor("out", [ROWS, Kn], F32, kind="ExternalOutput")

    with TileContext(nc) as tc:
        with (
            tc.tile_pool(name="const", bufs=1) as constp,
            tc.tile_pool(name="ops", bufs=1) as opp,
            tc.tile_pool(name="gsb", bufs=2) as gsbp,
            tc.tile_pool(name="ext", bufs=1) as extp,
            tc.tile_pool(name="fmp", bufs=1) as fmp,
            tc.tile_pool(name="psA", bufs=2, space="PSUM") as psA,
            tc.tile_pool(name="psB", bufs=2, space="PSUM") as psB,
        ):
            sel_t = constp.tile([GW, R * CW], BF16)
            nc.sync.dma_start(out=sel_t[:], in_=sel_d[:])
            meta_t = constp.tile([ROWS, Kn], F32)
            nc.sync.dma_start(out=meta_t[:], in_=meta_d[:])

            # staging for rows: ext [7, (r 16)(g 8)(t 2)(c 7)] per pair
            exts = [
                extp.tile([CW, R * GROUPS * 2 * CW], F32, name=f"extp{p}")
                for p in range(Kn // 2)
            ]
            final_t = fmp.tile([ROWS, Kn * NSTAT], F32)

            # PE pstate warmup: keep PE busy during the first load so the
            # ramp to full clock completes before the first gram matmul.
            nwarm = cfg.get("warmup", 100)
            if nwarm:
                wv = psB.tile([128, 1024], F32, tag="ps2")
                for i in range(nwarm):
                    nc.tensor.matmul(
                        wv[0:CW, 0:CW], sel_t[:, 0:CW], sel_t[:, 0:CW],
                        start=True, stop=True, skip_group_check=True,
                    )

            op_t = []
            for t in range(Kn):
                op = opp.tile([ROWS, chunks[t] * GROUPS * GW], op_dt, name=f"op{t}")
                half = (GROUPS // 2) * chunks[t] * GW
                nc.sync.dma_start(out=op[:, 0:half], in_=op_d[t][:, 0:half])
                nc.sync.dma_start(out=op[:, half:], in_=op_d[t][:, half:])
                op_t.append(op)

            out_t = fmp.tile([ROWS, Kn], F32)
            grams = {}
            gsbs = {}
            evs = {}
            fvv = final_t[:].rearrange("p (t k c) -> p t k c", t=Kn, k=CW)

            def emit_grams(t):
                Ct = chunks[t]
                op = op_t[t]
                gram = psA.tile([128, 1024], F32, tag="gram")
                gv = gram[:].rearrange("p (g w) -> p g w", g=GROUPS)
                use_dr = cfg.get("double_row", True) and op_dt == FP8
                for g in range(GROUPS):
                    if use_dr:
                        npair = Ct // 2
                        for c in range(npair):
                            sl = op[
                                :, (g * Ct + 2 * c) * GW : (g * Ct + 2 * c + 2) * GW
                            ].rearrange("p (k w) -> p k w", k=2)
                            nc.tensor.matmul(
                                gv[0:GW, g, 0:GW], sl, sl,
                                start=(c == 0), stop=(c == npair - 1 and Ct % 2 == 0),
                                skip_group_check=True,
                                perf_mode=mybir.MatmulPerfMode.DoubleRow,
                            )
                        if Ct % 2:
                            sl = op[:, (g * Ct + Ct - 1) * GW : (g * Ct + Ct) * GW]
                            nc.tensor.matmul(
                                gv[0:GW, g, 0:GW], sl, sl,
                                start=(Ct == 1), stop=True,
                                skip_group_check=True,
                            )
                    else:
                        for c in range(Ct):
                            sl = op[:, (g * Ct + c) * GW : (g * Ct + c + 1) * GW]
                            nc.tensor.matmul(
                                gv[0:GW, g, 0:GW], sl, sl,
                                start=(c == 0), stop=(c == Ct - 1),
                                skip_group_check=True,
                            )
                grams[t] = gv
                # Act copy1 queued immediately (runs when grams stop)
                gram_sb = gsbp.tile([GW, GROUPS * GW], BF16, tag="gramsb")
                gsv = gram_sb[:].rearrange("p (g w) -> p g w", g=GROUPS)
                nc.scalar.activation(gsv[:, :, :], gv[0:GW, :, 0:GW], AF.Copy)
                gsbs[t] = gsv

            def emit_selects(t):
                gsv = gsbs[t]
                ps2 = psB.tile([128, 1024], F32, tag="ps2")
                p2v = ps2[:].rearrange("p (r w) -> p r w", r=R)
                for r in range(R):
                    rhs = gsv[:, :, CW * r : CW * r + CW]
                    lhsT = sel_t[:, CW * r : CW * r + CW]
                    nc.tensor.matmul(
                        p2v[0:CW, r, 0 : GROUPS * CW], lhsT, rhs,
                        start=True, stop=True, skip_group_check=True,
                    )
                pair, tp = divmod(t, 2)
                ev = exts[pair][:].rearrange(
                    "p (r g t c) -> p r g t c", r=R, g=GROUPS, t=2
                )
                nc.scalar.activation(
                    ev[:, :, :, tp, :],
                    p2v[0:CW, :, 0 : GROUPS * CW].rearrange(
                        "p r (g c) -> p r g c", g=GROUPS
                    ),
                    AF.Copy,
                )
                evs[pair] = ev

            def emit_finals(pair):
                ev = evs[pair]
                for kkc in range(CW):
                    eng = nc.sync if kkc % 2 == 0 else nc.gpsimd
                    eng.dma_start(
                        out=fvv[:, 2 * pair : 2 * pair + 2, kkc, :],
                        in_=ev[kkc : kkc + 1, :, :, :, :],
                    )

            if do_mm and do_extract:
                emit_grams(0)
                emit_grams(1)
                emit_selects(0)
                emit_grams(2)
                emit_selects(1)
                emit_finals(0)
                if do_math and cfg.get("twopass", False):
                    fm = _FM(nc, fmp, 2, prefix="m0_")
                    _emit_math_pair(
                        nc, fm, final_t, meta_t[:, 0:2], out_t[:, 0:2], Kn, 0, 2
                    )
                emit_grams(3)
                emit_selects(2)
                emit_selects(3)
                emit_finals(1)
                if do_math and cfg.get("twopass", False):
                    fm = _FM(nc, fmp, 2, prefix="m1_")
                    _emit_math_pair(
                        nc, fm, final_t, meta_t[:, 2:4], out_t[:, 2:4], Kn, 2, 2
                    )
                elif do_math:
                    fm = _FM(nc, fmp, Kn, prefix="m_")
                    _emit_math_pair(
                        nc, fm, final_t, meta_t[:], out_t[:], Kn, 0, Kn
                    )
            elif do_mm:
                for t in range(Kn):
                    emit_grams(t)
            if not (do_mm and do_extract and do_math):
                nc.vector.memset(out_t[:], 0.0)
            nc.sync.dma_start(out=out_d[:], in_=out_t[:])
    return nc


# ---------------------------------------------------------------------------
# Host side
# ---------------------------------------------------------------------------
def plan_shards(num_atoms, n_classes=K):
    B = num_atoms.shape[0]
    assert B % (N_CORES * ROWS) == 0
    assert n_classes == B // (N_CORES * ROWS)
    order = np.argsort(num_atoms, kind="stable")
    na_sorted = num_atoms[order]
    rows_per_class = N_CORES * ROWS
    chunks = []
    for k in range(n_classes):
        mx = int(na_sorted[(k + 1) * rows_per_class - 1])
        chunks.append((mx + CHUNK - 1) // CHUNK)
    chunks = chunks[::-1]  # tile t=0 = biggest class
    return order, chunks


def _pack_tile(x, y, na, Ct):
    """x, y: [128, nmax, 3] f32 (row-major positions), na: [128] int.
    Returns op [128, Ct, GROUPS, GW] f32 with atoms on dim 0 (partitions)."""
    nmax = x.shape[1]
    cap = Ct * CHUNK
    # data [b, n, 7]
    d = np.zeros((ROWS, cap, CW), np.float32)
    ncl = min(cap, nmax)
    d[:, :ncl, 0:3] = x[:, :ncl, :]
    d[:, :ncl, 3:6] = y[:, :ncl, :]
    mask = (np.arange(cap)[None, :] < na[:, None]).astype(np.float32)
    d[:, :, 0:6] *= mask[:, :, None]
    d[:, :, 6] = 1.0
    # op[p, g, c, 7r+k] = d[8r+g, c*128+p, k]   (group-major for strip loads)
    d = d.reshape(ROWS, Ct, CHUNK, CW)            # [b, c, p, k]
    d = d.transpose(2, 1, 0, 3)                   # [p, c, b, k]
    d = d.reshape(CHUNK, Ct, R, GROUPS, CW)       # [p, c, r, g, k]  (b = 8r+g)
    d = d.transpose(0, 3, 1, 2, 4)                # [p, g, c, r, k]
    return np.ascontiguousarray(d.reshape(CHUNK, GROUPS, Ct, GW))


def _op_np_dtype():
    return mybir.dt.np(OP_DT)


def shard_inputs(coords_input, coords_target, num_atoms, order, chunks):
    import ml_dtypes

    B, ncols = coords_input.shape
    nmax = ncols // 3
    Kn = len(chunks)
    rows_per_class = N_CORES * ROWS
    sel = np.zeros((GW, R * CW), np.float32)
    for j in range(R * CW):
        sel[j, j] = 1.0
    sel = sel.astype(ml_dtypes.bfloat16)
    in_maps = []
    core_row_idx = []
    for c in range(N_CORES):
        m = {"sel": sel}
        idx_all = []
        meta = np.zeros((ROWS, Kn), np.float32)
        for t in range(Kn):
            kcls = Kn - 1 - t  # tile t=0 = biggest class
            idx = order[kcls * rows_per_class + c * ROWS : kcls * rows_per_class + (c + 1) * ROWS]
            idx_all.append(idx)
            na = num_atoms[idx]
            meta[:, t] = na.astype(np.float32)
            x = coords_input[idx].reshape(ROWS, nmax, 3)
            y = coords_target[idx].reshape(ROWS, nmax, 3)
            op = _pack_tile(x, y, na, chunks[t])
            m[f"op{t}"] = np.ascontiguousarray(
                op.reshape(CHUNK, -1)
            ).astype(_op_np_dtype())
        m["meta"] = meta
        in_maps.append(m)
        core_row_idx.append(np.concatenate(idx_all))
    return in_maps, core_row_idx


def unshard_outputs(results, core_row_idx, B):
    out = np.empty(B, dtype=np.float32)
    for c in range(N_CORES):
        o = results[c]["out"]  # [ROWS, K]
        out[core_row_idx[c]] = o.T.reshape(-1)
    return out


# ---------------------------------------------------------------------------
# Entry point
# ---------------------------------------------------------------------------
_PROG_CACHE = {}


def _get_program(chunks):
    key = tuple(chunks)
    if key not in _PROG_CACHE:
        _PROG_CACHE[key] = build_program(list(chunks))
    return _PROG_CACHE[key]


def kernel(coords_input, coords_target, num_atoms):
    from concourse.bass_utils import run_bass_kernel_spmd

    x = np.ascontiguousarray(np.asarray(coords_input, dtype=np.float32))
    y = np.ascontiguousarray(np.asarray(coords_target, dtype=np.float32))
    na = np.asarray(num_atoms).astype(np.int64)
    B, ncols = x.shape
    Kn = B // (N_CORES * ROWS)
    assert B == N_CORES * ROWS * Kn, f"unsupported batch {B}"

    order, chunks = plan_shards(na, n_classes=Kn)
    in_maps, core_row_idx = shard_inputs(x, y, na, order, chunks)
    nc = _get_program(chunks)
    res = run_bass_kernel_spmd(nc, in_maps, core_ids=list(range(N_CORES)))
    out = unshard_outputs(res.results, core_row_idx, B)
    return out.astype(np.float32)


# revision 26
# speedup vs baseline: 2.5026x; 1.0157x over previous
"""Bass/Trainium2 kernel for batched masked-Kabsch RMSD (nn_Coords2RMSD).

PE-centric design (SPMD across 8 cores):
  - Host sorts rows by num_atoms into 4 size classes (32 sorted tiles of
    128 rows; core c takes one tile per class). Per tile, coords are
    repacked TRANSPOSED: atoms on SBUF partitions, and for each group of
    16 rows a 112-column operand [x0 x1 x2 y0 y1 y2 1] per row (7 cols
    x 16 rows). Padding atoms are zeroed on the host; the ones column
    makes the Gram matrix carry the masked sums.
  - Per (group, chunk-of-128-atoms) ONE symmetric PE matmul
    op^T @ op accumulates into PSUM: diag 7x7 blocks per row hold all 21
    statistics (cross-covariance, |x|^2, |y|^2, sums) at once.
  - Extraction: Act copies PSUM->SBUF (bf16), 16 identity-select matmuls
    gather the diagonal blocks into a second PSUM, Act copies them into a
    row-indexed staging buffer, and 7 strided DMAs per tile-pair
    transpose [slot-comp, row] -> [row, stats].
  - Final math (centroid correction, 3x3 C^T C eigenvalues via the
    closed-form trig method, Kabsch det sign, RMSD) runs on [128, 4]
    fp32 tiles, one column per class.
"""

import numpy as np

import concourse.bass as bass
import concourse.mybir as mybir
from concourse.tile import TileContext, ScopedClock

F32 = mybir.dt.float32
BF16 = mybir.dt.bfloat16
FP8 = mybir.dt.float8e4
OP_DT = FP8  # gram operand dtype (host-cast)
OP = mybir.AluOpType
AF = mybir.ActivationFunctionType

N_CORES = 8
ROWS = 128          # rows per tile == final partitions
GROUPS = 8          # row-groups per tile
R = 16              # rows per group
CW = 7              # cols per row: x0 x1 x2 y0 y1 y2 1
GW = R * CW         # group operand width = 112
CHUNK = 128         # atoms per matmul pass (contraction dim)
NSTAT = CW * CW     # 49 stats per row
K = 4               # classes (tiles per core)


# ---------------------------------------------------------------------------
# TileContext tail patch: this walrus build accepts at most ONE sync-wait
# command per instruction and no sem-eq waits, so the stock drain + EVSEM
# butterfly fails codegen. Emit a ge-wait-only tail instead.
# ---------------------------------------------------------------------------
def _patched_drain_and_barrier(self, tick_clock, wait_clock):
    nc = self.nc
    dummy = nc.gpsimd.nop()
    wait_clock.add_sem_waits(dummy.ins, ScopedClock({None: tick_clock.global_clock}))
    waits = list(dummy.ins.sync_info.on_wait) if dummy.ins.sync_info else []
    if dummy.ins.sync_info:
        dummy.ins.sync_info = mybir.SyncInfo(on_wait=[], on_update=[])

    bsem = nc.alloc_semaphore(f"tail_bsem_{nc.next_id()}")
    dsem = nc.alloc_semaphore(f"tail_dsem_{nc.next_id()}")
    n_eng = 0
    for eng in nc.engines.values():
        eng.drain()
        eng.sem_inc(bsem, 1)
        n_eng += 1
    nc.gpsimd.wait_ge(bsem, n_eng)
    for w in waits:
        n = nc.gpsimd.nop()
        n.ins.sync_info = mybir.SyncInfo(on_wait=[w], on_update=[])
    nc.gpsimd.sem_inc(dsem, 1)
    for eng in nc.engines.values():
        if eng is not nc.gpsimd:
            eng.wait_ge(dsem, 1)

    popped = nc._tile_sem_poison_stack.pop()
    assert popped is self._sem_poison
    nc.clear_and_free_semaphores(list(self.sems.allocated().values()))
    nc.gpsimd.sem_clear(bsem)
    nc.gpsimd.sem_clear(dsem)


def install_tile_patch():
    TileContext._drain_and_barrier = _patched_drain_and_barrier


# ---------------------------------------------------------------------------
# BIR post-pass: split multi-wait sync infos onto NoOps (walrus accepts at
# most one sync-wait command per instruction, none on Drain).
# ---------------------------------------------------------------------------
_orig_to_json_bytes = bass.Bass.to_json_bytes


def _split_multiwait_json(self) -> bytes:
    import json

    raw = _orig_to_json_bytes(self)
    m = json.loads(raw)
    ctr = 0
    changed = False
    for f in m.get("functions", []):
        for blk in f.get("blocks", []):
            insts = blk.get("instructions", [])
            out = []
            for inst in insts:
                si = inst.get("sync_info")
                ow = (si or {}).get("on_wait") or []
                opc = str(inst.get("opcode", inst.get("type", "")))
                limit = 0 if opc == "Drain" else 1
                if len(ow) > limit:
                    keep = ow[len(ow) - limit :] if limit else []
                    moved = ow[: len(ow) - limit] if limit else ow
                    for w in moved:
                        ctr += 1
                        out.append(
                            {
                                "debug": inst.get("debug", 0),
                                "engine": inst["engine"],
                                "ins": [],
                                "name": f"WS-{ctr}-{inst['name']}",
                                "opcode": "NoOp",
                                "outs": [],
                                "sync_info": {"on_update": [], "on_wait": [w]},
                            }
                        )
                    si["on_wait"] = keep
                    changed = True
                out.append(inst)
            blk["instructions"] = out
    if not changed:
        return raw
    return json.dumps(m).encode()


bass.Bass.to_json_bytes = _split_multiwait_json


# ---------------------------------------------------------------------------
# Final math emitter on [128, K] fp32 column tiles.
# final layout: [128 rows, (t: K)(kk: 7)(cc: 7)] fp32
#   G(kk, cc) = sum_n op[n, kk] op[n, cc] per row (kk,cc in 0..5 = comps,
#   6 = ones => sums). Columns for class t at offset t*49.
# ---------------------------------------------------------------------------
class _FM:
    def __init__(self, nc, pool, Kn, prefix=""):
        self.nc = nc
        self.pool = pool
        self.K = Kn
        self.n = 0
        self.prefix = prefix
        self._consts = {}

    def const_col(self, val):
        val = float(val)
        if val in self._consts:
            return self._consts[val]
        i = len(self._consts)
        t = self.pool.tile([ROWS, 1], F32, tag=f"fmc{i}", name=f"fmc{i}")
        self.nc.vector.memset(t[:], val)
        self._consts[val] = t[:]
        return t[:]

    def t(self, w=None):
        self.n += 1
        nm = f"fm{self.prefix}{self.n}"
        return self.pool.tile([ROWS, w or self.K], F32, tag=nm, name=nm)

    def tt(self, a, b, op):
        o = self.t()
        self.nc.vector.tensor_tensor(o[:], a, b, op)
        return o[:]

    def mul(self, a, b):
        return self.tt(a, b, OP.mult)

    def add(self, a, b):
        return self.tt(a, b, OP.add)

    def sub(self, a, b):
        return self.tt(a, b, OP.subtract)

    def ts(self, a, s, op):
        o = self.t()
        self.nc.vector.tensor_scalar(o[:], a, float(s), None, op)
        return o[:]

    def ts2(self, a, s1, s2, op0, op1):
        o = self.t()
        self.nc.vector.tensor_scalar(o[:], a, float(s1), float(s2), op0, op1)
        return o[:]

    def stt(self, a, s, b, op0, op1):
        """(a op0 s) op1 b"""
        o = self.t()
        self.nc.vector.scalar_tensor_tensor(o[:], a, float(s), b, op0, op1)
        return o[:]

    def act(self, a, func, bias=0.0, scale=1.0):
        o = self.t()
        if isinstance(bias, float) and bias not in (0.0, 1.0) and func != AF.Copy:
            bias = self.const_col(bias)
        self.nc.scalar.activation(o[:], a, func, bias=bias, scale=scale)
        return o[:]

    def recip(self, a):
        o = self.t()
        self.nc.vector.reciprocal(o[:], a)
        return o[:]


def _emit_math_pair(nc, fm, final_t, meta_ap, out_ap, Kn, t0, Kp):
    """Wide-op final math for classes [t0, t0+Kp)."""
    fv = final_t[:].rearrange("p (t k c) -> p t k c", t=Kn, k=CW)[
        :, t0 : t0 + Kp, :, :
    ]
    fvf = final_t[:].rearrange("p (t c) -> p t c", t=Kn)[
        :, t0 : t0 + Kp, :
    ]

    def W(w):  # fresh wide tile
        return fm.t(w)

    rn = fm.recip(meta_ap)  # [128, Kp]
    rn_b3 = rn[:, :, None].broadcast_to([ROWS, Kp, 3])

    P = fv[:, :, 0:3, 3:6]          # [128, Kp, 3, 3]
    Sall = fv[:, :, 6, 0:6]         # [128, Kp, 6]
    Sy = fv[:, :, 6, 3:6]
    rn_b6 = rn[:, :, None].broadcast_to([ROWS, Kp, 6])

    sn_t = W(Kp * 6)
    sn6 = sn_t[:].rearrange("p (t c) -> p t c", t=Kp)
    nc.vector.tensor_tensor(sn6, Sall, rn_b6, OP.mult)
    sxn = sn6[:, :, 0:3]

    t1_t = W(Kp * 9)
    t1 = t1_t[:].rearrange("p (t i j) -> p t i j", t=Kp, i=3)
    nc.vector.tensor_tensor(
        t1, sxn[:, :, :, None].broadcast_to([ROWS, Kp, 3, 3]),
        Sy[:, :, None, :].broadcast_to([ROWS, Kp, 3, 3]), OP.mult)
    C_t = W(Kp * 9)
    C = C_t[:].rearrange("p (t i j) -> p t i j", t=Kp, i=3)
    nc.vector.tensor_tensor(C, P, t1, OP.subtract)

    def Cij(i, j):
        return C[:, :, i, j]

    # M = C^T C via 3 outer products
    M_t = W(Kp * 9)
    M = M_t[:].rearrange("p (t a b) -> p t a b", t=Kp, a=3)
    tmp_t = W(Kp * 9)
    tmp = tmp_t[:].rearrange("p (t a b) -> p t a b", t=Kp, a=3)
    for i in range(3):
        Ci = C[:, :, i, :]
        dst = M if i == 0 else tmp
        nc.vector.tensor_tensor(
            dst, Ci[:, :, :, None].broadcast_to([ROWS, Kp, 3, 3]),
            Ci[:, :, None, :].broadcast_to([ROWS, Kp, 3, 3]), OP.mult)
        if i > 0:
            nc.vector.tensor_tensor(M, M, tmp, OP.add)

    Mf = M_t[:].rearrange("p (t ab) -> p t ab", t=Kp)
    Mdiag = Mf[:, :, 0:9:4]  # [128, 2, 3]

    # q = trM/3
    q = fm.add(Mdiag[:, :, 0], Mdiag[:, :, 1])
    q = fm.stt(Mdiag[:, :, 2], 1.0, q, OP.mult, OP.add)
    q = fm.ts(q, 1.0 / 3.0, OP.mult)

    # trM2 = sum M*M ; p2 = trM2 - 3 q^2
    MM_t = W(Kp * 9)
    nc.vector.tensor_tensor(MM_t[:], M_t[:], M_t[:], OP.mult)
    trM2 = fm.t()
    nc.vector.tensor_reduce(
        trM2[:], MM_t[:].rearrange("p (t ab) -> p t ab", t=Kp),
        mybir.AxisListType.X, OP.add)
    qq = fm.mul(q, q)
    p2 = fm.stt(qq, -3.0, trM2[:], OP.mult, OP.add)
    p2c = fm.ts2(p2, 1.0 / 6.0, 1e-20, OP.mult, OP.max)
    p = fm.ts(p2c, 0.5, OP.pow)

    # --- detC, detC^2, sign (DVE; pool per-op overhead hurts the chain) ---
    def gtt(a, b, op):
        o = fm.t()
        nc.vector.tensor_tensor(o[:], a, b, op)
        return o[:]

    gm0 = gtt(Cij(1, 1), Cij(2, 2), OP.mult)
    gm0b = gtt(Cij(1, 2), Cij(2, 1), OP.mult)
    gm0 = gtt(gm0, gm0b, OP.subtract)
    gm1 = gtt(Cij(1, 0), Cij(2, 2), OP.mult)
    gm1b = gtt(Cij(1, 2), Cij(2, 0), OP.mult)
    gm1 = gtt(gm1, gm1b, OP.subtract)
    gm2 = gtt(Cij(1, 0), Cij(2, 1), OP.mult)
    gm2b = gtt(Cij(1, 1), Cij(2, 0), OP.mult)
    gm2 = gtt(gm2, gm2b, OP.subtract)
    d0 = gtt(Cij(0, 0), gm0, OP.mult)
    d1 = gtt(Cij(0, 1), gm1, OP.mult)
    d2 = gtt(Cij(0, 2), gm2, OP.mult)
    detC = gtt(gtt(d0, d1, OP.subtract), d2, OP.add)
    detC2 = gtt(detC, detC, OP.mult)
    dneg = fm.t()
    nc.vector.tensor_scalar(dneg[:], detC, 0.0, None, OP.is_lt)

    # detKq = det(M - qI) = -2.5 q^3 + 0.5 q trM2 + detC^2
    q3 = fm.mul(qq, q)
    a_ = fm.mul(q, trM2[:])
    t_ = fm.stt(a_, 0.5, detC2, OP.mult, OP.add)
    detKq = fm.stt(q3, -2.5, t_, OP.mult, OP.add)

    # r = 0.5 detKq * p2c^-1.5 clamped
    z = fm.ts(p2c, -1.5, OP.pow)
    r = fm.stt(detKq, 0.5, z, OP.mult, OP.mult)
    r = fm.ts2(r, 1.0, -1.0, OP.min, OP.max)

    # Newton on 4c^3-3c=r for c1 (cos(phi)) and c3 (cos(phi+2pi/3)), packed
    # cubic init c1 = E(r^2) + r O(r^2); c3(r) = -c1(-r) = -E + r O
    E1, E0 = -0.07910172, 0.87011722
    O1, O0 = 0.06293734, 0.15509478
    rr = fm.mul(r, r)
    cpack_t = W(2 * Kp)
    cpack = cpack_t[:].rearrange("p (s t) -> p s t", s=2)
    Ev = fm.ts2(rr, E1, E0, OP.mult, OP.add)
    Ov = fm.ts2(rr, O1, O0, OP.mult, OP.add)
    rO = fm.mul(r, Ov)
    nc.vector.tensor_tensor(cpack[:, 0, :], Ev, rO, OP.add)
    nc.vector.tensor_tensor(cpack[:, 1, :], rO, Ev, OP.subtract)
    r_b = r[:, None, :].broadcast_to([ROWS, 2, Kp])
    for _ in range(2):
        c2 = fm.t(2 * Kp)
        nc.vector.tensor_tensor(c2[:], cpack_t[:], cpack_t[:], OP.mult)
        c3 = fm.t(2 * Kp)
        nc.vector.tensor_tensor(c3[:], c2[:], cpack_t[:], OP.mult)
        num = fm.t(2 * Kp)
        nc.vector.scalar_tensor_tensor(
            num[:].rearrange("p (s t) -> p s t", s=2),
            c3[:].rearrange("p (s t) -> p s t", s=2), 8.0, r_b,
            OP.mult, OP.add)
        den = fm.t(2 * Kp)
        nc.vector.tensor_scalar(den[:], c2[:], 12.0, -3.0, OP.mult, OP.add)
        nc.vector.tensor_tensor(cpack_t[:], num[:], den[:], OP.divide)

    # lambdas: l1 = q + 2p c1 ; l3 = q + 2p c3 ; l2 = 3q - l1 - l3
    p2x = fm.ts(p, 2.0, OP.mult)
    lpack_t = W(3 * Kp)
    lpack = lpack_t[:].rearrange("p (s t) -> p s t", s=3)
    p2x_b = p2x[:, None, :].broadcast_to([ROWS, 2, Kp])
    q_b = q[:, None, :].broadcast_to([ROWS, 2, Kp])
    tl_t = W(2 * Kp)
    tl = tl_t[:].rearrange("p (s t) -> p s t", s=2)
    nc.vector.tensor_tensor(tl, p2x_b, cpack, OP.mult)
    nc.vector.tensor_tensor(lpack[:, 0:2, :], q_b, tl, OP.add)
    t_l2 = fm.stt(q, 3.0, lpack[:, 0, :], OP.mult, OP.subtract)
    nc.vector.tensor_tensor(lpack[:, 2, :], t_l2, lpack[:, 1, :], OP.subtract)
    spack_t = fm.t(3 * Kp)
    nc.vector.tensor_scalar(spack_t[:], lpack_t[:], 0.0, 0.5, OP.max, OP.pow)
    spack = spack_t[:].rearrange("p (s t) -> p s t", s=3)

    tr = fm.add(fm.add(spack[:, 0, :], spack[:, 2, :]), spack[:, 1, :])
    tr = fm.stt(fm.mul(dneg[:], spack[:, 1, :]), -2.0, tr, OP.mult, OP.add)

    # gx + gy: one reduce over all six diag cols; packed sum-sq reduce
    Qsum = fm.t()
    nc.vector.tensor_reduce(Qsum[:], fvf[:, :, 0:41:8], mybir.AxisListType.X, OP.add)
    snS_t = W(Kp * 6)
    nc.vector.tensor_tensor(
        snS_t[:].rearrange("p (t c) -> p t c", t=Kp), sn6, Sall, OP.mult)
    s2sum = fm.t()
    nc.vector.tensor_reduce(
        s2sum[:], snS_t[:].rearrange("p (t c) -> p t c", t=Kp),
        mybir.AxisListType.X, OP.add)
    g = fm.sub(Qsum[:], s2sum[:])

    diff = fm.stt(tr, -2.0, g, OP.mult, OP.add)
    msd = fm.mul(diff, rn)
    nc.vector.tensor_scalar(out_ap, msd, 0.0, 0.5, OP.max, OP.pow)


# ---------------------------------------------------------------------------
# Program builder
# ---------------------------------------------------------------------------
def build_program(chunks, cfg=None):
    """chunks: per-class chunk counts (len K). Returns nc."""
    cfg = cfg or {}
    do_mm = cfg.get("mm", True)
    do_extract = cfg.get("extract", True)
    do_math = cfg.get("math", True)
    Kn = len(chunks)
    install_tile_patch()
    nc = bass.Bass()
    op_dt = FP8 if cfg.get("fp8", True) else BF16
    op_d = [
        nc.dram_tensor(f"op{t}", [ROWS, chunks[t] * GROUPS * GW], op_dt,
                       kind="ExternalInput")
        for t in range(Kn)
    ]
    sel_d = nc.dram_tensor("sel", [GW, R * CW], BF16, kind="ExternalInput")
    meta_d = nc.dram_tensor("meta", [ROWS, Kn], F32, kind="ExternalInput")
    out_d = nc.dram_tensor("out", [ROWS, Kn], F32, kind="ExternalOutput")

    with TileContext(nc) as tc:
        with (
            tc.tile_pool(name="const", bufs=1) as constp,
            tc.tile_pool(name="ops", bufs=1) as opp,
            tc.tile_pool(name="gsb", bufs=2) as gsbp,
            tc.tile_pool(name="ext", bufs=1) as extp,
            tc.tile_pool(name="fmp", bufs=1) as fmp,
            tc.tile_pool(name="psA", bufs=2, space="PSUM") as psA,
            tc.tile_pool(name="psB", bufs=2, space="PSUM") as psB,
        ):
            sel_t = constp.tile([GW, R * CW], BF16)
            nc.sync.dma_start(out=sel_t[:], in_=sel_d[:])
            meta_t = constp.tile([ROWS, Kn], F32)
            nc.sync.dma_start(out=meta_t[:], in_=meta_d[:])

            # staging for rows: ext [7, (r 16)(g 8)(t 2)(c 7)] per pair
            exts = [
                extp.tile([CW, R * GROUPS * 2 * CW], F32, name=f"extp{p}")
                for p in range(Kn // 2)
            ]
            final_t = fmp.tile([ROWS, Kn * NSTAT], F32)

            # PE pstate warmup: keep PE busy during the first load so the
            # ramp to full clock completes before the first gram matmul.
            nwarm = cfg.get("warmup", 100)
            if nwarm:
                wv = psB.tile([128, 1024], F32, tag="ps2")
                for i in range(nwarm):
                    nc.tensor.matmul(
                        wv[0:CW, 0:CW], sel_t[:, 0:CW], sel_t[:, 0:CW],
                        start=True, stop=True, skip_group_check=True,
                    )

            op_t = []
            for t in range(Kn):
                op = opp.tile([ROWS, chunks[t] * GROUPS * GW], op_dt, name=f"op{t}")
                half = (GROUPS // 2) * chunks[t] * GW
                nc.sync.dma_start(out=op[:, 0:half], in_=op_d[t][:, 0:half])
                nc.sync.dma_start(out=op[:, half:], in_=op_d[t][:, half:])
                op_t.append(op)

            out_t = fmp.tile([ROWS, Kn], F32)
            grams = {}
            gsbs = {}
            evs = {}
            fvv = final_t[:].rearrange("p (t k c) -> p t k c", t=Kn, k=CW)

            def emit_grams(t):
                Ct = chunks[t]
                op = op_t[t]
                gram = psA.tile([128, 1024], F32, tag="gram")
                gv = gram[:].rearrange("p (g w) -> p g w", g=GROUPS)
                use_dr = cfg.get("double_row", True) and op_dt == FP8
                for g in range(GROUPS):
                    if use_dr:
                        npair = Ct // 2
                        for c in range(npair):
                            sl = op[
                                :, (g * Ct + 2 * c) * GW : (g * Ct + 2 * c + 2) * GW
                            ].rearrange("p (k w) -> p k w", k=2)
                            nc.tensor.matmul(
                                gv[0:GW, g, 0:GW], sl, sl,
                                start=(c == 0), stop=(c == npair - 1 and Ct % 2 == 0),
                                skip_group_check=True,
                                perf_mode=mybir.MatmulPerfMode.DoubleRow,
                            )
                        if Ct % 2:
                            sl = op[:, (g * Ct + Ct - 1) * GW : (g * Ct + Ct) * GW]
                            nc.tensor.matmul(
                                gv[0:GW, g, 0:GW], sl, sl,
                                start=(Ct == 1), stop=True,
                                skip_group_check=True,
                            )
                    else:
                        for c in range(Ct):
                            sl = op[:, (g * Ct + c) * GW : (g * Ct + c + 1) * GW]
                            nc.tensor.matmul(
                                gv[0:GW, g, 0:GW], sl, sl,
                                start=(c == 0), stop=(c == Ct - 1),
                                skip_group_check=True,
                            )
                grams[t] = gv
                # Act copy1 queued immediately (runs when grams stop)
                gram_sb = gsbp.tile([GW, GROUPS * GW], BF16, tag="gramsb")
                gsv = gram_sb[:].rearrange("p (g w) -> p g w", g=GROUPS)
                nc.scalar.activation(gsv[:, :, :], gv[0:GW, :, 0:GW], AF.Copy)
                gsbs[t] = gsv

            def emit_selects(t):
                gsv = gsbs[t]
                ps2 = psB.tile([128, 1024], F32, tag="ps2")
                p2v = ps2[:].rearrange("p (r w) -> p r w", r=R)
                for r in range(R):
                    rhs = gsv[:, :, CW * r : CW * r + CW]
                    lhsT = sel_t[:, CW * r : CW * r + CW]
                    nc.tensor.matmul(
                        p2v[0:CW, r, 0 : GROUPS * CW], lhsT, rhs,
                        start=True, stop=True, skip_group_check=True,
                    )
                pair, tp = divmod(t, 2)
                ev = exts[pair][:].rearrange(
                    "p (r g t c) -> p r g t c", r=R, g=GROUPS, t=2
                )
                nc.scalar.activation(
                    ev[:, :, :, tp, :],
                    p2v[0:CW, :, 0 : GROUPS * CW].rearrange(
                        "p r (g c) -> p r g c", g=GROUPS
                    ),
                    AF.Copy,
                )
                evs[pair] = ev

            def emit_finals(t, both=False):
                pair, tp = divmod(t, 2)
                ev = evs[pair]
                for kkc in range(CW):
                    eng = nc.gpsimd if kkc in (2, 5) else nc.sync
                    if both:
                        eng.dma_start(
                            out=fvv[:, 2 * pair : 2 * pair + 2, kkc, :],
                            in_=ev[kkc : kkc + 1, :, :, :, :],
                        )
                    else:
                        eng.dma_start(
                            out=fvv[:, t : t + 1, kkc, :],
                            in_=ev[kkc : kkc + 1, :, :, tp : tp + 1, :],
                        )

            if do_mm and do_extract:
                emit_grams(0)
                emit_grams(1)
                emit_selects(0)
                emit_grams(2)
                emit_selects(1)
                emit_finals(1, both=True)
                emit_grams(3)
                emit_selects(2)
                emit_selects(3)
                emit_finals(3, both=True)
                if do_math:
                    fm = _FM(nc, fmp, Kn, prefix="m_")
                    _emit_math_pair(
                        nc, fm, final_t, meta_t[:], out_t[:], Kn, 0, Kn
                    )
            elif do_mm:
                for t in range(Kn):
                    emit_grams(t)
            if not (do_mm and do_extract and do_math):
                nc.vector.memset(out_t[:], 0.0)
            nc.sync.dma_start(out=out_d[:], in_=out_t[:])
    return nc


# ---------------------------------------------------------------------------
# Host side
# ---------------------------------------------------------------------------
def plan_shards(num_atoms, n_classes=K):
    B = num_atoms.shape[0]
    assert B % (N_CORES * ROWS) == 0
    assert n_classes == B // (N_CORES * ROWS)
    order = np.argsort(num_atoms, kind="stable")
    na_sorted = num_atoms[order]
    rows_per_class = N_CORES * ROWS
    chunks = []
    for k in range(n_classes):
        mx = int(na_sorted[(k + 1) * rows_per_class - 1])
        chunks.append((mx + CHUNK - 1) // CHUNK)
    chunks = chunks[::-1]  # tile t=0 = biggest class
    return order, chunks


def _pack_tile(x, y, na, Ct):
    """x, y: [128, nmax, 3] f32 (row-major positions), na: [128] int.
    Returns op [128, Ct, GROUPS, GW] f32 with atoms on dim 0 (partitions)."""
    nmax = x.shape[1]
    cap = Ct * CHUNK
    # data [b, n, 7]
    d = np.zeros((ROWS, cap, CW), np.float32)
    ncl = min(cap, nmax)
    d[:, :ncl, 0:3] = x[:, :ncl, :]
    d[:, :ncl, 3:6] = y[:, :ncl, :]
    mask = (np.arange(cap)[None, :] < na[:, None]).astype(np.float32)
    d[:, :, 0:6] *= mask[:, :, None]
    d[:, :, 6] = 1.0
    # op[p, g, c, 7r+k] = d[8r+g, c*128+p, k]   (group-major for strip loads)
    d = d.reshape(ROWS, Ct, CHUNK, CW)            # [b, c, p, k]
    d = d.transpose(2, 1, 0, 3)                   # [p, c, b, k]
    d = d.reshape(CHUNK, Ct, R, GROUPS, CW)       # [p, c, r, g, k]  (b = 8r+g)
    d = d.transpose(0, 3, 1, 2, 4)                # [p, g, c, r, k]
    return np.ascontiguousarray(d.reshape(CHUNK, GROUPS, Ct, GW))


def _op_np_dtype():
    return mybir.dt.np(OP_DT)


def shard_inputs(coords_input, coords_target, num_atoms, order, chunks):
    import ml_dtypes

    B, ncols = coords_input.shape
    nmax = ncols // 3
    Kn = len(chunks)
    rows_per_class = N_CORES * ROWS
    sel = np.zeros((GW, R * CW), np.float32)
    for j in range(R * CW):
        sel[j, j] = 1.0
    sel = sel.astype(ml_dtypes.bfloat16)
    in_maps = []
    core_row_idx = []
    for c in range(N_CORES):
        m = {"sel": sel}
        idx_all = []
        meta = np.zeros((ROWS, Kn), np.float32)
        for t in range(Kn):
            kcls = Kn - 1 - t  # tile t=0 = biggest class
            idx = order[kcls * rows_per_class + c * ROWS : kcls * rows_per_class + (c + 1) * ROWS]
            idx_all.append(idx)
            na = num_atoms[idx]
            meta[:, t] = na.astype(np.float32)
            x = coords_input[idx].reshape(ROWS, nmax, 3)
            y = coords_target[idx].reshape(ROWS, nmax, 3)
            op = _pack_tile(x, y, na, chunks[t])
            m[f"op{t}"] = np.ascontiguousarray(
                op.reshape(CHUNK, -1)
            ).astype(_op_np_dtype())
        m["meta"] = meta
        in_maps.append(m)
        core_row_idx.append(np.concatenate(idx_all))
    return in_maps, core_row_idx


def unshard_outputs(results, core_row_idx, B):
    out = np.empty(B, dtype=np.float32)
    for c in range(N_CORES):
        o = results[c]["out"]  # [ROWS, K]
        out[core_row_idx[c]] = o.T.reshape(-1)
    return out


# ---------------------------------------------------------------------------
# Entry point
# ---------------------------------------------------------------------------
_PROG_CACHE = {}


def _get_program(chunks):
    key = tuple(chunks)
    if key not in _PROG_CACHE:
        _PROG_CACHE[key] = build_program(list(chunks))
    return _PROG_CACHE[key]


def kernel(coords_input, coords_target, num_atoms):
    from concourse.bass_utils import run_bass_kernel_spmd

    x = np.ascontiguousarray(np.asarray(coords_input, dtype=np.float32))
    y = np.ascontiguousarray(np.asarray(coords_target, dtype=np.float32))
    na = np.asarray(num_atoms).astype(np.int64)
    B, ncols = x.shape
    Kn = B // (N_CORES * ROWS)
    assert B == N_CORES * ROWS * Kn, f"unsupported batch {B}"

    order, chunks = plan_shards(na, n_classes=Kn)
    in_maps, core_row_idx = shard_inputs(x, y, na, order, chunks)
    nc = _get_program(chunks)
    res = run_bass_kernel_spmd(nc, in_maps, core_ids=list(range(N_CORES)))
    out = unshard_outputs(res.results, core_row_idx, B)
    return out.astype(np.float32)


# revision 29
# speedup vs baseline: 2.5331x; 1.0122x over previous
"""Bass/Trainium2 kernel for batched masked-Kabsch RMSD (nn_Coords2RMSD).

PE-centric design (SPMD across 8 cores):
  - Host sorts rows by num_atoms into 4 size classes (32 sorted tiles of
    128 rows; core c takes one tile per class). Per tile, coords are
    repacked TRANSPOSED: atoms on SBUF partitions, and for each group of
    16 rows a 112-column operand [x0 x1 x2 y0 y1 y2 1] per row (7 cols
    x 16 rows). Padding atoms are zeroed on the host; the ones column
    makes the Gram matrix carry the masked sums.
  - Per (group, chunk-of-128-atoms) ONE symmetric PE matmul
    op^T @ op accumulates into PSUM: diag 7x7 blocks per row hold all 21
    statistics (cross-covariance, |x|^2, |y|^2, sums) at once.
  - Extraction: Act copies PSUM->SBUF (bf16), 16 identity-select matmuls
    gather the diagonal blocks into a second PSUM, Act copies them into a
    row-indexed staging buffer, and 7 strided DMAs per tile-pair
    transpose [slot-comp, row] -> [row, stats].
  - Final math (centroid correction, 3x3 C^T C eigenvalues via the
    closed-form trig method, Kabsch det sign, RMSD) runs on [128, 4]
    fp32 tiles, one column per class.
"""

import numpy as np

import concourse.bass as bass
import concourse.mybir as mybir
from concourse.tile import TileContext, ScopedClock

F32 = mybir.dt.float32
BF16 = mybir.dt.bfloat16
FP8 = mybir.dt.float8e4
OP_DT = FP8  # gram operand dtype (host-cast)
OP = mybir.AluOpType
AF = mybir.ActivationFunctionType

N_CORES = 8
ROWS = 128          # rows per tile == final partitions
GROUPS = 8          # row-groups per tile
R = 16              # rows per group
CW = 7              # cols per row: x0 x1 x2 y0 y1 y2 1
GW = R * CW         # group operand width = 112
CHUNK = 128         # atoms per matmul pass (contraction dim)
NSTAT = CW * CW     # 49 stats per row
K = 4               # classes (tiles per core)


# ---------------------------------------------------------------------------
# TileContext tail patch: this walrus build accepts at most ONE sync-wait
# command per instruction and no sem-eq waits, so the stock drain + EVSEM
# butterfly fails codegen. Emit a ge-wait-only tail instead.
# ---------------------------------------------------------------------------
def _patched_drain_and_barrier(self, tick_clock, wait_clock):
    nc = self.nc
    dummy = nc.gpsimd.nop()
    wait_clock.add_sem_waits(dummy.ins, ScopedClock({None: tick_clock.global_clock}))
    waits = list(dummy.ins.sync_info.on_wait) if dummy.ins.sync_info else []
    if dummy.ins.sync_info:
        dummy.ins.sync_info = mybir.SyncInfo(on_wait=[], on_update=[])

    bsem = nc.alloc_semaphore(f"tail_bsem_{nc.next_id()}")
    dsem = nc.alloc_semaphore(f"tail_dsem_{nc.next_id()}")
    n_eng = 0
    for eng in nc.engines.values():
        eng.drain()
        eng.sem_inc(bsem, 1)
        n_eng += 1
    nc.gpsimd.wait_ge(bsem, n_eng)
    for w in waits:
        n = nc.gpsimd.nop()
        n.ins.sync_info = mybir.SyncInfo(on_wait=[w], on_update=[])
    nc.gpsimd.sem_inc(dsem, 1)
    for eng in nc.engines.values():
        if eng is not nc.gpsimd:
            eng.wait_ge(dsem, 1)

    popped = nc._tile_sem_poison_stack.pop()
    assert popped is self._sem_poison
    nc.clear_and_free_semaphores(list(self.sems.allocated().values()))
    nc.gpsimd.sem_clear(bsem)
    nc.gpsimd.sem_clear(dsem)


def install_tile_patch():
    TileContext._drain_and_barrier = _patched_drain_and_barrier


# ---------------------------------------------------------------------------
# BIR post-pass: split multi-wait sync infos onto NoOps (walrus accepts at
# most one sync-wait command per instruction, none on Drain).
# ---------------------------------------------------------------------------
_orig_to_json_bytes = bass.Bass.to_json_bytes


def _split_multiwait_json(self) -> bytes:
    import json

    raw = _orig_to_json_bytes(self)
    m = json.loads(raw)
    ctr = 0
    changed = False
    for f in m.get("functions", []):
        for blk in f.get("blocks", []):
            insts = blk.get("instructions", [])
            out = []
            for inst in insts:
                si = inst.get("sync_info")
                ow = (si or {}).get("on_wait") or []
                opc = str(inst.get("opcode", inst.get("type", "")))
                limit = 0 if opc == "Drain" else 1
                if len(ow) > limit:
                    keep = ow[len(ow) - limit :] if limit else []
                    moved = ow[: len(ow) - limit] if limit else ow
                    for w in moved:
                        ctr += 1
                        out.append(
                            {
                                "debug": inst.get("debug", 0),
                                "engine": inst["engine"],
                                "ins": [],
                                "name": f"WS-{ctr}-{inst['name']}",
                                "opcode": "NoOp",
                                "outs": [],
                                "sync_info": {"on_update": [], "on_wait": [w]},
                            }
                        )
                    si["on_wait"] = keep
                    changed = True
                out.append(inst)
            blk["instructions"] = out
    if not changed:
        return raw
    return json.dumps(m).encode()


bass.Bass.to_json_bytes = _split_multiwait_json


# ---------------------------------------------------------------------------
# Final math emitter on [128, K] fp32 column tiles.
# final layout: [128 rows, (t: K)(kk: 7)(cc: 7)] fp32
#   G(kk, cc) = sum_n op[n, kk] op[n, cc] per row (kk,cc in 0..5 = comps,
#   6 = ones => sums). Columns for class t at offset t*49.
# ---------------------------------------------------------------------------
class _FM:
    def __init__(self, nc, pool, Kn, prefix=""):
        self.nc = nc
        self.pool = pool
        self.K = Kn
        self.n = 0
        self.prefix = prefix
        self._consts = {}

    def const_col(self, val):
        val = float(val)
        if val in self._consts:
            return self._consts[val]
        i = len(self._consts)
        t = self.pool.tile([ROWS, 1], F32, tag=f"fmc{i}", name=f"fmc{i}")
        self.nc.vector.memset(t[:], val)
        self._consts[val] = t[:]
        return t[:]

    def t(self, w=None):
        self.n += 1
        nm = f"fm{self.prefix}{self.n}"
        return self.pool.tile([ROWS, w or self.K], F32, tag=nm, name=nm)

    def tt(self, a, b, op):
        o = self.t()
        self.nc.vector.tensor_tensor(o[:], a, b, op)
        return o[:]

    def mul(self, a, b):
        return self.tt(a, b, OP.mult)

    def add(self, a, b):
        return self.tt(a, b, OP.add)

    def sub(self, a, b):
        return self.tt(a, b, OP.subtract)

    def ts(self, a, s, op):
        o = self.t()
        self.nc.vector.tensor_scalar(o[:], a, float(s), None, op)
        return o[:]

    def ts2(self, a, s1, s2, op0, op1):
        o = self.t()
        self.nc.vector.tensor_scalar(o[:], a, float(s1), float(s2), op0, op1)
        return o[:]

    def stt(self, a, s, b, op0, op1):
        """(a op0 s) op1 b"""
        o = self.t()
        self.nc.vector.scalar_tensor_tensor(o[:], a, float(s), b, op0, op1)
        return o[:]

    def act(self, a, func, bias=0.0, scale=1.0):
        o = self.t()
        if isinstance(bias, float) and bias not in (0.0, 1.0) and func != AF.Copy:
            bias = self.const_col(bias)
        self.nc.scalar.activation(o[:], a, func, bias=bias, scale=scale)
        return o[:]

    def recip(self, a):
        o = self.t()
        self.nc.vector.reciprocal(o[:], a)
        return o[:]


def _emit_math_pair(nc, fm, final_t, meta_ap, out_ap, Kn, t0, Kp):
    """Wide-op final math for classes [t0, t0+Kp)."""
    fv = final_t[:].rearrange("p (t k c) -> p t k c", t=Kn, k=CW)[
        :, t0 : t0 + Kp, :, :
    ]
    fvf = final_t[:].rearrange("p (t c) -> p t c", t=Kn)[
        :, t0 : t0 + Kp, :
    ]

    def W(w):  # fresh wide tile
        return fm.t(w)

    rn = fm.recip(meta_ap)  # [128, Kp]
    rn_b3 = rn[:, :, None].broadcast_to([ROWS, Kp, 3])

    P = fv[:, :, 0:3, 3:6]          # [128, Kp, 3, 3]
    Sall = fv[:, :, 6, 0:6]         # [128, Kp, 6]
    Sy = fv[:, :, 6, 3:6]
    rn_b6 = rn[:, :, None].broadcast_to([ROWS, Kp, 6])

    sn_t = W(Kp * 6)
    sn6 = sn_t[:].rearrange("p (t c) -> p t c", t=Kp)
    nc.vector.tensor_tensor(sn6, Sall, rn_b6, OP.mult)
    sxn = sn6[:, :, 0:3]

    t1_t = W(Kp * 9)
    t1 = t1_t[:].rearrange("p (t i j) -> p t i j", t=Kp, i=3)
    nc.vector.tensor_tensor(
        t1, sxn[:, :, :, None].broadcast_to([ROWS, Kp, 3, 3]),
        Sy[:, :, None, :].broadcast_to([ROWS, Kp, 3, 3]), OP.mult)
    C_t = W(Kp * 9)
    C = C_t[:].rearrange("p (t i j) -> p t i j", t=Kp, i=3)
    nc.vector.tensor_tensor(C, P, t1, OP.subtract)

    def Cij(i, j):
        return C[:, :, i, j]

    # M = C^T C via 3 outer products
    M_t = W(Kp * 9)
    M = M_t[:].rearrange("p (t a b) -> p t a b", t=Kp, a=3)
    tmp_t = W(Kp * 9)
    tmp = tmp_t[:].rearrange("p (t a b) -> p t a b", t=Kp, a=3)
    for i in range(3):
        Ci = C[:, :, i, :]
        dst = M if i == 0 else tmp
        nc.vector.tensor_tensor(
            dst, Ci[:, :, :, None].broadcast_to([ROWS, Kp, 3, 3]),
            Ci[:, :, None, :].broadcast_to([ROWS, Kp, 3, 3]), OP.mult)
        if i > 0:
            nc.vector.tensor_tensor(M, M, tmp, OP.add)

    Mf = M_t[:].rearrange("p (t ab) -> p t ab", t=Kp)
    Mdiag = Mf[:, :, 0:9:4]  # [128, 2, 3]

    # q = trM/3
    q = fm.add(Mdiag[:, :, 0], Mdiag[:, :, 1])
    q = fm.stt(Mdiag[:, :, 2], 1.0, q, OP.mult, OP.add)
    q = fm.ts(q, 1.0 / 3.0, OP.mult)

    # trM2 = sum M*M ; p2 = trM2 - 3 q^2
    MM_t = W(Kp * 9)
    nc.vector.tensor_tensor(MM_t[:], M_t[:], M_t[:], OP.mult)
    trM2 = fm.t()
    nc.vector.tensor_reduce(
        trM2[:], MM_t[:].rearrange("p (t ab) -> p t ab", t=Kp),
        mybir.AxisListType.X, OP.add)
    qq = fm.mul(q, q)
    p2 = fm.stt(qq, -3.0, trM2[:], OP.mult, OP.add)
    p2c = fm.ts2(p2, 1.0 / 6.0, 1e-30, OP.mult, OP.max)
    p = fm.act(p2c, AF.Sqrt)

    # --- detC, detC^2, sign (DVE; pool per-op overhead hurts the chain) ---
    def gtt(a, b, op):
        o = fm.t()
        nc.vector.tensor_tensor(o[:], a, b, op)
        return o[:]

    gm0 = gtt(Cij(1, 1), Cij(2, 2), OP.mult)
    gm0b = gtt(Cij(1, 2), Cij(2, 1), OP.mult)
    gm0 = gtt(gm0, gm0b, OP.subtract)
    gm1 = gtt(Cij(1, 0), Cij(2, 2), OP.mult)
    gm1b = gtt(Cij(1, 2), Cij(2, 0), OP.mult)
    gm1 = gtt(gm1, gm1b, OP.subtract)
    gm2 = gtt(Cij(1, 0), Cij(2, 1), OP.mult)
    gm2b = gtt(Cij(1, 1), Cij(2, 0), OP.mult)
    gm2 = gtt(gm2, gm2b, OP.subtract)
    d0 = gtt(Cij(0, 0), gm0, OP.mult)
    d1 = gtt(Cij(0, 1), gm1, OP.mult)
    d2 = gtt(Cij(0, 2), gm2, OP.mult)
    detC = gtt(gtt(d0, d1, OP.subtract), d2, OP.add)
    detC2 = gtt(detC, detC, OP.mult)
    dneg = fm.t()
    nc.vector.tensor_scalar(dneg[:], detC, 0.0, None, OP.is_lt)

    # detKq = det(M - qI) = -2.5 q^3 + 0.5 q trM2 + detC^2
    q3 = fm.mul(qq, q)
    a_ = fm.mul(q, trM2[:])
    t_ = fm.stt(a_, 0.5, detC2, OP.mult, OP.add)
    detKq = fm.stt(q3, -2.5, t_, OP.mult, OP.add)

    # r = 0.5 detKq / p^3 clamped
    rp = fm.recip(p)
    rp3 = fm.mul(fm.mul(rp, rp), rp)
    r = fm.stt(detKq, 0.5, rp3, OP.mult, OP.mult)
    r = fm.ts2(r, 1.0, -1.0, OP.min, OP.max)

    # Newton on 4c^3-3c=r for c1 (cos(phi)) and c3 (cos(phi+2pi/3)), packed
    # cubic init c1 = E(r^2) + r O(r^2); c3(r) = -c1(-r) = -E + r O
    E1, E0 = -0.07910172, 0.87011722
    O1, O0 = 0.06293734, 0.15509478
    rr = fm.mul(r, r)
    cpack_t = W(2 * Kp)
    cpack = cpack_t[:].rearrange("p (s t) -> p s t", s=2)
    Ev = fm.ts2(rr, E1, E0, OP.mult, OP.add)
    Ov = fm.ts2(rr, O1, O0, OP.mult, OP.add)
    rO = fm.mul(r, Ov)
    nc.vector.tensor_tensor(cpack[:, 0, :], Ev, rO, OP.add)
    nc.vector.tensor_tensor(cpack[:, 1, :], rO, Ev, OP.subtract)
    r_b = r[:, None, :].broadcast_to([ROWS, 2, Kp])
    for _ in range(2):
        c2 = fm.t(2 * Kp)
        nc.vector.tensor_tensor(c2[:], cpack_t[:], cpack_t[:], OP.mult)
        c3 = fm.t(2 * Kp)
        nc.vector.tensor_tensor(c3[:], c2[:], cpack_t[:], OP.mult)
        num = fm.t(2 * Kp)
        nc.vector.scalar_tensor_tensor(
            num[:].rearrange("p (s t) -> p s t", s=2),
            c3[:].rearrange("p (s t) -> p s t", s=2), 8.0, r_b,
            OP.mult, OP.add)
        den = fm.t(2 * Kp)
        nc.vector.tensor_scalar(den[:], c2[:], 12.0, -3.0, OP.mult, OP.add)
        rec = fm.t(2 * Kp)
        nc.vector.reciprocal(rec[:], den[:])
        nc.vector.tensor_tensor(cpack_t[:], num[:], rec[:], OP.mult)

    # lambdas: l1 = q + 2p c1 ; l3 = q + 2p c3 ; l2 = 3q - l1 - l3
    p2x = fm.ts(p, 2.0, OP.mult)
    lpack_t = W(3 * Kp)
    lpack = lpack_t[:].rearrange("p (s t) -> p s t", s=3)
    p2x_b = p2x[:, None, :].broadcast_to([ROWS, 2, Kp])
    q_b = q[:, None, :].broadcast_to([ROWS, 2, Kp])
    tl_t = W(2 * Kp)
    tl = tl_t[:].rearrange("p (s t) -> p s t", s=2)
    nc.vector.tensor_tensor(tl, p2x_b, cpack, OP.mult)
    nc.vector.tensor_tensor(lpack[:, 0:2, :], q_b, tl, OP.add)
    t_l2 = fm.stt(q, 3.0, lpack[:, 0, :], OP.mult, OP.subtract)
    nc.vector.tensor_tensor(lpack[:, 2, :], t_l2, lpack[:, 1, :], OP.subtract)
    lmax = fm.t(3 * Kp)
    nc.vector.tensor_scalar(lmax[:], lpack_t[:], 0.0, None, OP.max)
    spack_t = fm.t(3 * Kp)
    nc.scalar.activation(spack_t[:], lmax[:], AF.Sqrt)
    spack = spack_t[:].rearrange("p (s t) -> p s t", s=3)

    tr = fm.add(fm.add(spack[:, 0, :], spack[:, 2, :]), spack[:, 1, :])
    tr = fm.stt(fm.mul(dneg[:], spack[:, 1, :]), -2.0, tr, OP.mult, OP.add)

    # gx + gy: one reduce over all six diag cols; packed sum-sq reduce
    Qsum = fm.t()
    nc.vector.tensor_reduce(Qsum[:], fvf[:, :, 0:41:8], mybir.AxisListType.X, OP.add)
    snS_t = W(Kp * 6)
    nc.vector.tensor_tensor(
        snS_t[:].rearrange("p (t c) -> p t c", t=Kp), sn6, Sall, OP.mult)
    s2sum = fm.t()
    nc.vector.tensor_reduce(
        s2sum[:], snS_t[:].rearrange("p (t c) -> p t c", t=Kp),
        mybir.AxisListType.X, OP.add)
    g = fm.sub(Qsum[:], s2sum[:])

    diff = fm.stt(tr, -2.0, g, OP.mult, OP.add)
    msd = fm.mul(diff, rn)
    nc.scalar.activation(out_ap, fm.ts(msd, 0.0, OP.max), AF.Sqrt)


# ---------------------------------------------------------------------------
# Program builder
# ---------------------------------------------------------------------------
def build_program(chunks, cfg=None):
    """chunks: per-class chunk counts (len K). Returns nc."""
    cfg = cfg or {}
    do_mm = cfg.get("mm", True)
    do_extract = cfg.get("extract", True)
    do_math = cfg.get("math", True)
    Kn = len(chunks)
    install_tile_patch()
    nc = bass.Bass()
    op_dt = FP8 if cfg.get("fp8", True) else BF16
    op_d = [
        nc.dram_tensor(f"op{t}", [ROWS, chunks[t] * GROUPS * GW], op_dt,
                       kind="ExternalInput")
        for t in range(Kn)
    ]
    sel_d = nc.dram_tensor("sel", [GW, R * CW], BF16, kind="ExternalInput")
    meta_d = nc.dram_tensor("meta", [ROWS, Kn], F32, kind="ExternalInput")
    out_d = nc.dram_tensor("out", [ROWS, Kn], F32, kind="ExternalOutput")

    with TileContext(nc) as tc:
        with (
            tc.tile_pool(name="const", bufs=1) as constp,
            tc.tile_pool(name="ops", bufs=1) as opp,
            tc.tile_pool(name="gsb", bufs=2) as gsbp,
            tc.tile_pool(name="ext", bufs=1) as extp,
            tc.tile_pool(name="fmp", bufs=1) as fmp,
            tc.tile_pool(name="psA", bufs=2, space="PSUM") as psA,
            tc.tile_pool(name="psB", bufs=2, space="PSUM") as psB,
        ):
            sel_t = constp.tile([GW, R * CW], BF16)
            nc.sync.dma_start(out=sel_t[:], in_=sel_d[:])
            meta_t = constp.tile([ROWS, Kn], F32)
            nc.sync.dma_start(out=meta_t[:], in_=meta_d[:])

            # staging for rows: ext [7, (r 16)(g 8)(t 2)(c 7)] per pair
            exts = [
                extp.tile([CW, R * GROUPS * 2 * CW], F32, name=f"extp{p}")
                for p in range(Kn // 2)
            ]
            final_t = fmp.tile([ROWS, Kn * NSTAT], F32)

            # PE pstate warmup: keep PE busy during the first load so the
            # ramp to full clock completes before the first gram matmul.
            nwarm = cfg.get("warmup", 100)
            if nwarm:
                wv = psB.tile([128, 1024], F32, tag="ps2")
                for i in range(nwarm):
                    nc.tensor.matmul(
                        wv[0:CW, 0:CW], sel_t[:, 0:CW], sel_t[:, 0:CW],
                        start=True, stop=True, skip_group_check=True,
                    )

            op_t = []
            for t in range(Kn):
                op = opp.tile([ROWS, chunks[t] * GROUPS * GW], op_dt, name=f"op{t}")
                half = (GROUPS // 2) * chunks[t] * GW
                nc.sync.dma_start(out=op[:, 0:half], in_=op_d[t][:, 0:half])
                nc.sync.dma_start(out=op[:, half:], in_=op_d[t][:, half:])
                op_t.append(op)

            out_t = fmp.tile([ROWS, Kn], F32)
            grams = {}
            gsbs = {}
            evs = {}
            fvv = final_t[:].rearrange("p (t k c) -> p t k c", t=Kn, k=CW)

            def emit_grams(t):
                Ct = chunks[t]
                op = op_t[t]
                gram = psA.tile([128, 1024], F32, tag="gram")
                gv = gram[:].rearrange("p (g w) -> p g w", g=GROUPS)
                use_dr = cfg.get("double_row", True) and op_dt == FP8
                for g in range(GROUPS):
                    if use_dr:
                        npair = Ct // 2
                        for c in range(npair):
                            sl = op[
                                :, (g * Ct + 2 * c) * GW : (g * Ct + 2 * c + 2) * GW
                            ].rearrange("p (k w) -> p k w", k=2)
                            nc.tensor.matmul(
                                gv[0:GW, g, 0:GW], sl, sl,
                                start=(c == 0), stop=(c == npair - 1 and Ct % 2 == 0),
                                skip_group_check=True,
                                perf_mode=mybir.MatmulPerfMode.DoubleRow,
                            )
                        if Ct % 2:
                            sl = op[:, (g * Ct + Ct - 1) * GW : (g * Ct + Ct) * GW]
                            nc.tensor.matmul(
                                gv[0:GW, g, 0:GW], sl, sl,
                                start=(Ct == 1), stop=True,
                                skip_group_check=True,
                            )
                    else:
                        for c in range(Ct):
                            sl = op[:, (g * Ct + c) * GW : (g * Ct + c + 1) * GW]
                            nc.tensor.matmul(
                                gv[0:GW, g, 0:GW], sl, sl,
                                start=(c == 0), stop=(c == Ct - 1),
                                skip_group_check=True,
                            )
                grams[t] = gv
                # Act copy1 queued immediately (runs when grams stop)
                gram_sb = gsbp.tile([GW, GROUPS * GW], BF16, tag="gramsb")
                gsv = gram_sb[:].rearrange("p (g w) -> p g w", g=GROUPS)
                nc.scalar.activation(gsv[:, :, :], gv[0:GW, :, 0:GW], AF.Copy)
                gsbs[t] = gsv

            def emit_selects(t):
                gsv = gsbs[t]
                ps2 = psB.tile([128, 1024], F32, tag="ps2")
                p2v = ps2[:].rearrange("p (r w) -> p r w", r=R)
                for r in range(R):
                    rhs = gsv[:, :, CW * r : CW * r + CW]
                    lhsT = sel_t[:, CW * r : CW * r + CW]
                    nc.tensor.matmul(
                        p2v[0:CW, r, 0 : GROUPS * CW], lhsT, rhs,
                        start=True, stop=True, skip_group_check=True,
                    )
                pair, tp = divmod(t, 2)
                ev = exts[pair][:].rearrange(
                    "p (r g t c) -> p r g t c", r=R, g=GROUPS, t=2
                )
                nc.scalar.activation(
                    ev[:, :, :, tp, :],
                    p2v[0:CW, :, 0 : GROUPS * CW].rearrange(
                        "p r (g c) -> p r g c", g=GROUPS
                    ),
                    AF.Copy,
                )
                evs[pair] = ev

            def emit_finals(t, both=False):
                pair, tp = divmod(t, 2)
                ev = evs[pair]
                for kkc in [6, 0, 1, 2, 3, 4, 5]:
                    eng = nc.gpsimd if kkc in (2, 5) else nc.sync
                    if both:
                        eng.dma_start(
                            out=fvv[:, 2 * pair : 2 * pair + 2, kkc, :],
                            in_=ev[kkc : kkc + 1, :, :, :, :],
                        )
                    else:
                        eng.dma_start(
                            out=fvv[:, t : t + 1, kkc, :],
                            in_=ev[kkc : kkc + 1, :, :, tp : tp + 1, :],
                        )

            if do_mm and do_extract:
                emit_grams(0)
                emit_grams(1)
                emit_selects(0)
                emit_grams(2)
                emit_selects(1)
                emit_finals(1, both=True)
                emit_grams(3)
                emit_selects(2)
                emit_selects(3)
                emit_finals(3, both=True)
                if do_math:
                    fm = _FM(nc, fmp, Kn, prefix="m_")
                    _emit_math_pair(
                        nc, fm, final_t, meta_t[:], out_t[:], Kn, 0, Kn
                    )
            elif do_mm:
                for t in range(Kn):
                    emit_grams(t)
            if not (do_mm and do_extract and do_math):
                nc.vector.memset(out_t[:], 0.0)
            nc.sync.dma_start(out=out_d[:], in_=out_t[:])
    return nc


# ---------------------------------------------------------------------------
# Host side
# ---------------------------------------------------------------------------
def plan_shards(num_atoms, n_classes=K):
    B = num_atoms.shape[0]
    assert B % (N_CORES * ROWS) == 0
    assert n_classes == B // (N_CORES * ROWS)
    order = np.argsort(num_atoms, kind="stable")
    na_sorted = num_atoms[order]
    rows_per_class = N_CORES * ROWS
    chunks = []
    for k in range(n_classes):
        mx = int(na_sorted[(k + 1) * rows_per_class - 1])
        chunks.append((mx + CHUNK - 1) // CHUNK)
    chunks = chunks[::-1]  # tile t=0 = biggest class
    return order, chunks


def _pack_tile(x, y, na, Ct):
    """x, y: [128, nmax, 3] f32 (row-major positions), na: [128] int.
    Returns op [128, Ct, GROUPS, GW] f32 with atoms on dim 0 (partitions)."""
    nmax = x.shape[1]
    cap = Ct * CHUNK
    # data [b, n, 7]
    d = np.zeros((ROWS, cap, CW), np.float32)
    ncl = min(cap, nmax)
    d[:, :ncl, 0:3] = x[:, :ncl, :]
    d[:, :ncl, 3:6] = y[:, :ncl, :]
    mask = (np.arange(cap)[None, :] < na[:, None]).astype(np.float32)
    d[:, :, 0:6] *= mask[:, :, None]
    d[:, :, 6] = 1.0
    # op[p, g, c, 7r+k] = d[8r+g, c*128+p, k]   (group-major for strip loads)
    d = d.reshape(ROWS, Ct, CHUNK, CW)            # [b, c, p, k]
    d = d.transpose(2, 1, 0, 3)                   # [p, c, b, k]
    d = d.reshape(CHUNK, Ct, R, GROUPS, CW)       # [p, c, r, g, k]  (b = 8r+g)
    d = d.transpose(0, 3, 1, 2, 4)                # [p, g, c, r, k]
    return np.ascontiguousarray(d.reshape(CHUNK, GROUPS, Ct, GW))


def _op_np_dtype():
    return mybir.dt.np(OP_DT)


def shard_inputs(coords_input, coords_target, num_atoms, order, chunks):
    import ml_dtypes

    B, ncols = coords_input.shape
    nmax = ncols // 3
    Kn = len(chunks)
    rows_per_class = N_CORES * ROWS
    sel = np.zeros((GW, R * CW), np.float32)
    for j in range(R * CW):
        sel[j, j] = 1.0
    sel = sel.astype(ml_dtypes.bfloat16)
    in_maps = []
    core_row_idx = []
    for c in range(N_CORES):
        m = {"sel": sel}
        idx_all = []
        meta = np.zeros((ROWS, Kn), np.float32)
        for t in range(Kn):
            kcls = Kn - 1 - t  # tile t=0 = biggest class
            idx = order[kcls * rows_per_class + c * ROWS : kcls * rows_per_class + (c + 1) * ROWS]
            idx_all.append(idx)
            na = num_atoms[idx]
            meta[:, t] = na.astype(np.float32)
            x = coords_input[idx].reshape(ROWS, nmax, 3)
            y = coords_target[idx].reshape(ROWS, nmax, 3)
            op = _pack_tile(x, y, na, chunks[t])
            m[f"op{t}"] = np.ascontiguousarray(
                op.reshape(CHUNK, -1)
            ).astype(_op_np_dtype())
        m["meta"] = meta
        in_maps.append(m)
        core_row_idx.append(np.concatenate(idx_all))
    return in_maps, core_row_idx


def unshard_outputs(results, core_row_idx, B):
    out = np.empty(B, dtype=np.float32)
    for c in range(N_CORES):
        o = results[c]["out"]  # [ROWS, K]
        out[core_row_idx[c]] = o.T.reshape(-1)
    return out


# ---------------------------------------------------------------------------
# Entry point
# ---------------------------------------------------------------------------
_PROG_CACHE = {}


def _get_program(chunks):
    key = tuple(chunks)
    if key not in _PROG_CACHE:
        _PROG_CACHE[key] = build_program(list(chunks))
    return _PROG_CACHE[key]


def kernel(coords_input, coords_target, num_atoms):
    from concourse.bass_utils import run_bass_kernel_spmd

    x = np.ascontiguousarray(np.asarray(coords_input, dtype=np.float32))
    y = np.ascontiguousarray(np.asarray(coords_target, dtype=np.float32))
    na = np.asarray(num_atoms).astype(np.int64)
    B, ncols = x.shape
    Kn = B // (N_CORES * ROWS)
    assert B == N_CORES * ROWS * Kn, f"unsupported batch {B}"

    order, chunks = plan_shards(na, n_classes=Kn)
    in_maps, core_row_idx = shard_inputs(x, y, na, order, chunks)
    nc = _get_program(chunks)
    res = run_bass_kernel_spmd(nc, in_maps, core_ids=list(range(N_CORES)))
    out = unshard_outputs(res.results, core_row_idx, B)
    return out.astype(np.float32)


# revision 31
# speedup vs baseline: 2.7878x; 1.1006x over previous
"""Bass/Trainium2 kernel for batched masked-Kabsch RMSD (nn_Coords2RMSD).

PE-centric design (SPMD across 8 cores):
  - Host sorts rows by num_atoms into 4 size classes (32 sorted tiles of
    128 rows; core c takes one tile per class). Per tile, coords are
    repacked TRANSPOSED: atoms on SBUF partitions, and for each group of
    16 rows a 112-column operand [x0 x1 x2 y0 y1 y2 1] per row (7 cols
    x 16 rows). Padding atoms are zeroed on the host; the ones column
    makes the Gram matrix carry the masked sums.
  - Per (group, chunk-of-128-atoms) ONE symmetric PE matmul
    op^T @ op accumulates into PSUM: diag 7x7 blocks per row hold all 21
    statistics (cross-covariance, |x|^2, |y|^2, sums) at once.
  - Extraction: Act copies PSUM->SBUF (bf16), 16 identity-select matmuls
    gather the diagonal blocks into a second PSUM, Act copies them into a
    row-indexed staging buffer, and 7 strided DMAs per tile-pair
    transpose [slot-comp, row] -> [row, stats].
  - Final math (centroid correction, 3x3 C^T C eigenvalues via the
    closed-form trig method, Kabsch det sign, RMSD) runs on [128, 4]
    fp32 tiles, one column per class.
"""

import numpy as np

import concourse.bass as bass
import concourse.mybir as mybir
from concourse.tile import TileContext, ScopedClock

F32 = mybir.dt.float32
BF16 = mybir.dt.bfloat16
FP8 = mybir.dt.float8e4
OP_DT = FP8  # gram operand dtype (host-cast)
OP = mybir.AluOpType
AF = mybir.ActivationFunctionType

N_CORES = 8
ROWS = 128          # rows per tile == final partitions
GROUPS = 8          # row-groups per tile
R = 16              # rows per group
CW = 7              # cols per row: x0 x1 x2 y0 y1 y2 1
GW = R * CW         # group operand width = 112
CHUNK = 128         # atoms per matmul pass (contraction dim)
NSTAT = CW * CW     # 49 stats per row
K = 4               # classes (tiles per core)


# ---------------------------------------------------------------------------
# TileContext tail patch: this walrus build accepts at most ONE sync-wait
# command per instruction and no sem-eq waits, so the stock drain + EVSEM
# butterfly fails codegen. Emit a ge-wait-only tail instead.
# ---------------------------------------------------------------------------
def _patched_drain_and_barrier(self, tick_clock, wait_clock):
    nc = self.nc
    dummy = nc.gpsimd.nop()
    wait_clock.add_sem_waits(dummy.ins, ScopedClock({None: tick_clock.global_clock}))
    waits = list(dummy.ins.sync_info.on_wait) if dummy.ins.sync_info else []
    if dummy.ins.sync_info:
        dummy.ins.sync_info = mybir.SyncInfo(on_wait=[], on_update=[])

    bsem = nc.alloc_semaphore(f"tail_bsem_{nc.next_id()}")
    dsem = nc.alloc_semaphore(f"tail_dsem_{nc.next_id()}")
    n_eng = 0
    for eng in nc.engines.values():
        eng.drain()
        eng.sem_inc(bsem, 1)
        n_eng += 1
    nc.gpsimd.wait_ge(bsem, n_eng)
    for w in waits:
        n = nc.gpsimd.nop()
        n.ins.sync_info = mybir.SyncInfo(on_wait=[w], on_update=[])
    nc.gpsimd.sem_inc(dsem, 1)
    for eng in nc.engines.values():
        if eng is not nc.gpsimd:
            eng.wait_ge(dsem, 1)

    popped = nc._tile_sem_poison_stack.pop()
    assert popped is self._sem_poison
    nc.clear_and_free_semaphores(list(self.sems.allocated().values()))
    nc.gpsimd.sem_clear(bsem)
    nc.gpsimd.sem_clear(dsem)


def install_tile_patch():
    TileContext._drain_and_barrier = _patched_drain_and_barrier


# ---------------------------------------------------------------------------
# BIR post-pass: split multi-wait sync infos onto NoOps (walrus accepts at
# most one sync-wait command per instruction, none on Drain).
# ---------------------------------------------------------------------------
_orig_to_json_bytes = bass.Bass.to_json_bytes


def _split_multiwait_json(self) -> bytes:
    import json

    raw = _orig_to_json_bytes(self)
    m = json.loads(raw)
    ctr = 0
    changed = False
    for f in m.get("functions", []):
        for blk in f.get("blocks", []):
            insts = blk.get("instructions", [])
            out = []
            for inst in insts:
                si = inst.get("sync_info")
                ow = (si or {}).get("on_wait") or []
                opc = str(inst.get("opcode", inst.get("type", "")))
                limit = 0 if opc == "Drain" else 1
                if len(ow) > limit:
                    keep = ow[len(ow) - limit :] if limit else []
                    moved = ow[: len(ow) - limit] if limit else ow
                    for w in moved:
                        ctr += 1
                        out.append(
                            {
                                "debug": inst.get("debug", 0),
                                "engine": inst["engine"],
                                "ins": [],
                                "name": f"WS-{ctr}-{inst['name']}",
                                "opcode": "NoOp",
                                "outs": [],
                                "sync_info": {"on_update": [], "on_wait": [w]},
                            }
                        )
                    si["on_wait"] = keep
                    changed = True
                out.append(inst)
            blk["instructions"] = out
    if not changed:
        return raw
    return json.dumps(m).encode()


bass.Bass.to_json_bytes = _split_multiwait_json


# ---------------------------------------------------------------------------
# Final math emitter on [128, K] fp32 column tiles.
# final layout: [128 rows, (t: K)(kk: 7)(cc: 7)] fp32
#   G(kk, cc) = sum_n op[n, kk] op[n, cc] per row (kk,cc in 0..5 = comps,
#   6 = ones => sums). Columns for class t at offset t*49.
# ---------------------------------------------------------------------------
class _FM:
    def __init__(self, nc, pool, Kn, prefix=""):
        self.nc = nc
        self.pool = pool
        self.K = Kn
        self.n = 0
        self.prefix = prefix
        self._consts = {}

    def const_col(self, val):
        val = float(val)
        if val in self._consts:
            return self._consts[val]
        i = len(self._consts)
        t = self.pool.tile([ROWS, 1], F32, tag=f"fmc{i}", name=f"fmc{i}")
        self.nc.vector.memset(t[:], val)
        self._consts[val] = t[:]
        return t[:]

    def t(self, w=None):
        self.n += 1
        nm = f"fm{self.prefix}{self.n}"
        return self.pool.tile([ROWS, w or self.K], F32, tag=nm, name=nm)

    def tt(self, a, b, op):
        o = self.t()
        self.nc.vector.tensor_tensor(o[:], a, b, op)
        return o[:]

    def mul(self, a, b):
        return self.tt(a, b, OP.mult)

    def add(self, a, b):
        return self.tt(a, b, OP.add)

    def sub(self, a, b):
        return self.tt(a, b, OP.subtract)

    def ts(self, a, s, op):
        o = self.t()
        self.nc.vector.tensor_scalar(o[:], a, float(s), None, op)
        return o[:]

    def ts2(self, a, s1, s2, op0, op1):
        o = self.t()
        self.nc.vector.tensor_scalar(o[:], a, float(s1), float(s2), op0, op1)
        return o[:]

    def stt(self, a, s, b, op0, op1):
        """(a op0 s) op1 b"""
        o = self.t()
        self.nc.vector.scalar_tensor_tensor(o[:], a, float(s), b, op0, op1)
        return o[:]

    def act(self, a, func, bias=0.0, scale=1.0):
        o = self.t()
        if isinstance(bias, float) and bias not in (0.0, 1.0) and func != AF.Copy:
            bias = self.const_col(bias)
        self.nc.scalar.activation(o[:], a, func, bias=bias, scale=scale)
        return o[:]

    def recip(self, a):
        o = self.t()
        self.nc.vector.reciprocal(o[:], a)
        return o[:]


def _emit_math_pair(nc, fm, final_t, meta_ap, out_ap, Kn, t0, Kp):
    """Wide-op final math for classes [t0, t0+Kp)."""
    fv = final_t[:].rearrange("p (t k c) -> p t k c", t=Kn, k=CW)[
        :, t0 : t0 + Kp, :, :
    ]
    fvf = final_t[:].rearrange("p (t c) -> p t c", t=Kn)[
        :, t0 : t0 + Kp, :
    ]

    def W(w):  # fresh wide tile
        return fm.t(w)

    rn = fm.recip(meta_ap)  # [128, Kp]
    rn_b3 = rn[:, :, None].broadcast_to([ROWS, Kp, 3])

    P = fv[:, :, 0:3, 3:6]          # [128, Kp, 3, 3]
    Sall = fv[:, :, 6, 0:6]         # [128, Kp, 6]
    Sy = fv[:, :, 6, 3:6]
    rn_b6 = rn[:, :, None].broadcast_to([ROWS, Kp, 6])

    sn_t = W(Kp * 6)
    sn6 = sn_t[:].rearrange("p (t c) -> p t c", t=Kp)
    nc.vector.tensor_tensor(sn6, Sall, rn_b6, OP.mult)
    sxn = sn6[:, :, 0:3]

    t1_t = W(Kp * 9)
    t1 = t1_t[:].rearrange("p (t i j) -> p t i j", t=Kp, i=3)
    nc.vector.tensor_tensor(
        t1, sxn[:, :, :, None].broadcast_to([ROWS, Kp, 3, 3]),
        Sy[:, :, None, :].broadcast_to([ROWS, Kp, 3, 3]), OP.mult)
    C_t = W(Kp * 9)
    C = C_t[:].rearrange("p (t i j) -> p t i j", t=Kp, i=3)
    nc.vector.tensor_tensor(C, P, t1, OP.subtract)

    def Cij(i, j):
        return C[:, :, i, j]

    # M = C^T C via 3 outer products
    M_t = W(Kp * 9)
    M = M_t[:].rearrange("p (t a b) -> p t a b", t=Kp, a=3)
    tmp_t = W(Kp * 9)
    tmp = tmp_t[:].rearrange("p (t a b) -> p t a b", t=Kp, a=3)
    for i in range(3):
        Ci = C[:, :, i, :]
        dst = M if i == 0 else tmp
        nc.vector.tensor_tensor(
            dst, Ci[:, :, :, None].broadcast_to([ROWS, Kp, 3, 3]),
            Ci[:, :, None, :].broadcast_to([ROWS, Kp, 3, 3]), OP.mult)
        if i > 0:
            nc.vector.tensor_tensor(M, M, tmp, OP.add)

    Mf = M_t[:].rearrange("p (t ab) -> p t ab", t=Kp)
    Mdiag = Mf[:, :, 0:9:4]  # [128, 2, 3]

    # q = trM/3
    q = fm.add(Mdiag[:, :, 0], Mdiag[:, :, 1])
    q = fm.stt(Mdiag[:, :, 2], 1.0, q, OP.mult, OP.add)
    q = fm.ts(q, 1.0 / 3.0, OP.mult)

    # trM2 = sum M*M ; p2 = trM2 - 3 q^2
    MM_t = W(Kp * 9)
    nc.vector.tensor_tensor(MM_t[:], M_t[:], M_t[:], OP.mult)
    trM2 = fm.t()
    nc.vector.tensor_reduce(
        trM2[:], MM_t[:].rearrange("p (t ab) -> p t ab", t=Kp),
        mybir.AxisListType.X, OP.add)
    qq = fm.mul(q, q)
    p2 = fm.stt(qq, -3.0, trM2[:], OP.mult, OP.add)
    p2c = fm.ts2(p2, 1.0 / 6.0, 1e-30, OP.mult, OP.max)
    p = fm.act(p2c, AF.Sqrt)

    # --- detC, detC^2, sign (DVE; pool per-op overhead hurts the chain) ---
    def gtt(a, b, op):
        o = fm.t()
        nc.vector.tensor_tensor(o[:], a, b, op)
        return o[:]

    gm0 = gtt(Cij(1, 1), Cij(2, 2), OP.mult)
    gm0b = gtt(Cij(1, 2), Cij(2, 1), OP.mult)
    gm0 = gtt(gm0, gm0b, OP.subtract)
    gm1 = gtt(Cij(1, 0), Cij(2, 2), OP.mult)
    gm1b = gtt(Cij(1, 2), Cij(2, 0), OP.mult)
    gm1 = gtt(gm1, gm1b, OP.subtract)
    gm2 = gtt(Cij(1, 0), Cij(2, 1), OP.mult)
    gm2b = gtt(Cij(1, 1), Cij(2, 0), OP.mult)
    gm2 = gtt(gm2, gm2b, OP.subtract)
    d0 = gtt(Cij(0, 0), gm0, OP.mult)
    d1 = gtt(Cij(0, 1), gm1, OP.mult)
    d2 = gtt(Cij(0, 2), gm2, OP.mult)
    detC = gtt(gtt(d0, d1, OP.subtract), d2, OP.add)
    detC2 = gtt(detC, detC, OP.mult)
    dneg = fm.t()
    nc.vector.tensor_scalar(dneg[:], detC, 0.0, None, OP.is_lt)

    # detKq = det(M - qI) = -2.5 q^3 + 0.5 q trM2 + detC^2
    q3 = fm.mul(qq, q)
    a_ = fm.mul(q, trM2[:])
    t_ = fm.stt(a_, 0.5, detC2, OP.mult, OP.add)
    detKq = fm.stt(q3, -2.5, t_, OP.mult, OP.add)

    # r = 0.5 detKq / p^3 clamped
    rp = fm.recip(p)
    rp3 = fm.mul(fm.mul(rp, rp), rp)
    r = fm.stt(detKq, 0.5, rp3, OP.mult, OP.mult)
    r = fm.ts2(r, 1.0, -1.0, OP.min, OP.max)

    # Newton on 4c^3-3c=r for c1 (cos(phi)) and c3 (cos(phi+2pi/3)), packed
    # cubic init c1 = E(r^2) + r O(r^2); c3(r) = -c1(-r) = -E + r O
    E1, E0 = -0.07910172, 0.87011722
    O1, O0 = 0.06293734, 0.15509478
    rr = fm.mul(r, r)
    cpack_t = W(2 * Kp)
    cpack = cpack_t[:].rearrange("p (s t) -> p s t", s=2)
    Ev = fm.ts2(rr, E1, E0, OP.mult, OP.add)
    Ov = fm.ts2(rr, O1, O0, OP.mult, OP.add)
    rO = fm.mul(r, Ov)
    nc.vector.tensor_tensor(cpack[:, 0, :], Ev, rO, OP.add)
    nc.vector.tensor_tensor(cpack[:, 1, :], rO, Ev, OP.subtract)
    r_b = r[:, None, :].broadcast_to([ROWS, 2, Kp])
    for _ in range(2):
        c2 = fm.t(2 * Kp)
        nc.vector.tensor_tensor(c2[:], cpack_t[:], cpack_t[:], OP.mult)
        c3 = fm.t(2 * Kp)
        nc.vector.tensor_tensor(c3[:], c2[:], cpack_t[:], OP.mult)
        num = fm.t(2 * Kp)
        nc.vector.scalar_tensor_tensor(
            num[:].rearrange("p (s t) -> p s t", s=2),
            c3[:].rearrange("p (s t) -> p s t", s=2), 8.0, r_b,
            OP.mult, OP.add)
        den = fm.t(2 * Kp)
        nc.vector.tensor_scalar(den[:], c2[:], 12.0, -3.0, OP.mult, OP.add)
        rec = fm.t(2 * Kp)
        nc.vector.reciprocal(rec[:], den[:])
        nc.vector.tensor_tensor(cpack_t[:], num[:], rec[:], OP.mult)

    # lambdas: l1 = q + 2p c1 ; l3 = q + 2p c3 ; l2 = 3q - l1 - l3
    p2x = fm.ts(p, 2.0, OP.mult)
    lpack_t = W(3 * Kp)
    lpack = lpack_t[:].rearrange("p (s t) -> p s t", s=3)
    p2x_b = p2x[:, None, :].broadcast_to([ROWS, 2, Kp])
    q_b = q[:, None, :].broadcast_to([ROWS, 2, Kp])
    tl_t = W(2 * Kp)
    tl = tl_t[:].rearrange("p (s t) -> p s t", s=2)
    nc.vector.tensor_tensor(tl, p2x_b, cpack, OP.mult)
    nc.vector.tensor_tensor(lpack[:, 0:2, :], q_b, tl, OP.add)
    t_l2 = fm.stt(q, 3.0, lpack[:, 0, :], OP.mult, OP.subtract)
    nc.vector.tensor_tensor(lpack[:, 2, :], t_l2, lpack[:, 1, :], OP.subtract)
    lmax = fm.t(3 * Kp)
    nc.vector.tensor_scalar(lmax[:], lpack_t[:], 0.0, None, OP.max)
    spack_t = fm.t(3 * Kp)
    nc.scalar.activation(spack_t[:], lmax[:], AF.Sqrt)
    spack = spack_t[:].rearrange("p (s t) -> p s t", s=3)

    # gx + gy: one reduce over all six diag cols; packed sum-sq reduce
    Qsum = fm.t()
    nc.vector.tensor_reduce(Qsum[:], fvf[:, :, 0:41:8], mybir.AxisListType.X, OP.add)
    snS_t = W(Kp * 6)
    nc.vector.tensor_tensor(
        snS_t[:].rearrange("p (t c) -> p t c", t=Kp), sn6, Sall, OP.mult)
    s2sum = fm.t()
    nc.vector.tensor_reduce(
        s2sum[:], snS_t[:].rearrange("p (t c) -> p t c", t=Kp),
        mybir.AxisListType.X, OP.add)
    g = fm.sub(Qsum[:], s2sum[:])
    tr = fm.add(fm.add(spack[:, 0, :], spack[:, 2, :]), spack[:, 1, :])
    tr = fm.stt(fm.mul(dneg[:], spack[:, 1, :]), -2.0, tr, OP.mult, OP.add)


    diff = fm.stt(tr, -2.0, g, OP.mult, OP.add)
    msd = fm.mul(diff, rn)
    nc.scalar.activation(out_ap, fm.ts(msd, 0.0, OP.max), AF.Sqrt)


# ---------------------------------------------------------------------------
# Program builder
# ---------------------------------------------------------------------------
def build_program(chunks, cfg=None):
    """chunks: per-class chunk counts (len K). Returns nc."""
    cfg = cfg or {}
    do_mm = cfg.get("mm", True)
    do_extract = cfg.get("extract", True)
    do_math = cfg.get("math", True)
    Kn = len(chunks)
    install_tile_patch()
    nc = bass.Bass()
    op_dt = FP8 if cfg.get("fp8", True) else BF16
    op_d = [
        nc.dram_tensor(f"op{t}", [ROWS, chunks[t] * GROUPS * GW], op_dt,
                       kind="ExternalInput")
        for t in range(Kn)
    ]
    sel_d = nc.dram_tensor("sel", [GW, R * CW], BF16, kind="ExternalInput")
    meta_d = nc.dram_tensor("meta", [ROWS, Kn], F32, kind="ExternalInput")
    out_d = nc.dram_tensor("out", [ROWS, Kn], F32, kind="ExternalOutput")

    with TileContext(nc) as tc:
        with (
            tc.tile_pool(name="const", bufs=1) as constp,
            tc.tile_pool(name="ops", bufs=1) as opp,
            tc.tile_pool(name="gsb", bufs=2) as gsbp,
            tc.tile_pool(name="ext", bufs=1) as extp,
            tc.tile_pool(name="fmp", bufs=1) as fmp,
            tc.tile_pool(name="psA", bufs=2, space="PSUM") as psA,
            tc.tile_pool(name="psB", bufs=2, space="PSUM") as psB,
        ):
            sel_t = constp.tile([GW, R * CW], BF16)
            nc.sync.dma_start(out=sel_t[:], in_=sel_d[:])
            meta_t = constp.tile([ROWS, Kn], F32)
            nc.sync.dma_start(out=meta_t[:], in_=meta_d[:])

            # staging for rows: ext [7, (r 16)(g 8)(t 2)(c 7)] per pair
            exts = [
                extp.tile([CW, R * GROUPS * 2 * CW], F32, name=f"extp{p}")
                for p in range(Kn // 2)
            ]
            final_t = fmp.tile([ROWS, Kn * NSTAT], F32)

            # PE pstate warmup: keep PE busy during the first load so the
            # ramp to full clock completes before the first gram matmul.
            nwarm = cfg.get("warmup", 100)
            if nwarm:
                wv = psB.tile([128, 1024], F32, tag="ps2")
                for i in range(nwarm):
                    nc.tensor.matmul(
                        wv[0:CW, 0:CW], sel_t[:, 0:CW], sel_t[:, 0:CW],
                        start=True, stop=True, skip_group_check=True,
                    )

            op_t = []
            for t in range(Kn):
                op = opp.tile([ROWS, chunks[t] * GROUPS * GW], op_dt, name=f"op{t}")
                half = (GROUPS // 2) * chunks[t] * GW
                nc.sync.dma_start(out=op[:, 0:half], in_=op_d[t][:, 0:half])
                nc.sync.dma_start(out=op[:, half:], in_=op_d[t][:, half:])
                op_t.append(op)

            out_t = fmp.tile([ROWS, Kn], F32)
            grams = {}
            gsbs = {}
            evs = {}
            fvv = final_t[:].rearrange("p (t k c) -> p t k c", t=Kn, k=CW)

            def emit_grams(t):
                Ct = chunks[t]
                op = op_t[t]
                gram = psA.tile([128, 1024], F32, tag="gram")
                gv = gram[:].rearrange("p (g w) -> p g w", g=GROUPS)
                use_dr = cfg.get("double_row", True) and op_dt == FP8
                for g in range(GROUPS):
                    if use_dr:
                        npair = Ct // 2
                        for c in range(npair):
                            sl = op[
                                :, (g * Ct + 2 * c) * GW : (g * Ct + 2 * c + 2) * GW
                            ].rearrange("p (k w) -> p k w", k=2)
                            nc.tensor.matmul(
                                gv[0:GW, g, 0:GW], sl, sl,
                                start=(c == 0), stop=(c == npair - 1 and Ct % 2 == 0),
                                skip_group_check=True,
                                perf_mode=mybir.MatmulPerfMode.DoubleRow,
                            )
                        if Ct % 2:
                            sl = op[:, (g * Ct + Ct - 1) * GW : (g * Ct + Ct) * GW]
                            nc.tensor.matmul(
                                gv[0:GW, g, 0:GW], sl, sl,
                                start=(Ct == 1), stop=True,
                                skip_group_check=True,
                            )
                    else:
                        for c in range(Ct):
                            sl = op[:, (g * Ct + c) * GW : (g * Ct + c + 1) * GW]
                            nc.tensor.matmul(
                                gv[0:GW, g, 0:GW], sl, sl,
                                start=(c == 0), stop=(c == Ct - 1),
                                skip_group_check=True,
                            )
                grams[t] = gv
                # Act copy1 queued immediately (runs when grams stop)
                gram_sb = gsbp.tile([GW, GROUPS * GW], BF16, tag="gramsb")
                gsv = gram_sb[:].rearrange("p (g w) -> p g w", g=GROUPS)
                nc.scalar.activation(gsv[:, :, :], gv[0:GW, :, 0:GW], AF.Copy)
                gsbs[t] = gsv

            def emit_selects(t):
                gsv = gsbs[t]
                ps2 = psB.tile([128, 1024], F32, tag="ps2")
                p2v = ps2[:].rearrange("p (r w) -> p r w", r=R)
                for r in range(R):
                    rhs = gsv[:, :, CW * r : CW * r + CW]
                    lhsT = sel_t[:, CW * r : CW * r + CW]
                    nc.tensor.matmul(
                        p2v[0:CW, r, 0 : GROUPS * CW], lhsT, rhs,
                        start=True, stop=True, skip_group_check=True,
                    )
                pair, tp = divmod(t, 2)
                ev = exts[pair][:].rearrange(
                    "p (r g t c) -> p r g t c", r=R, g=GROUPS, t=2
                )
                nc.scalar.activation(
                    ev[:, :, :, tp, :],
                    p2v[0:CW, :, 0 : GROUPS * CW].rearrange(
                        "p r (g c) -> p r g c", g=GROUPS
                    ),
                    AF.Copy,
                )
                evs[pair] = ev

            def emit_finals(t, both=False):
                pair, tp = divmod(t, 2)
                ev = evs[pair]
                for kkc in [6, 0, 1, 2, 3, 4, 5]:
                    eng = nc.gpsimd if kkc in (2, 5) else nc.sync
                    if both:
                        eng.dma_start(
                            out=fvv[:, 2 * pair : 2 * pair + 2, kkc, :],
                            in_=ev[kkc : kkc + 1, :, :, :, :],
                        )
                    else:
                        eng.dma_start(
                            out=fvv[:, t : t + 1, kkc, :],
                            in_=ev[kkc : kkc + 1, :, :, tp : tp + 1, :],
                        )

            if do_mm and do_extract:
                emit_grams(0)
                emit_grams(1)
                emit_selects(0)
                emit_grams(2)
                emit_selects(1)
                emit_finals(1, both=True)
                emit_grams(3)
                emit_selects(2)
                emit_selects(3)
                emit_finals(3, both=True)
                if do_math:
                    fm = _FM(nc, fmp, Kn, prefix="m_")
                    _emit_math_pair(
                        nc, fm, final_t, meta_t[:], out_t[:], Kn, 0, Kn
                    )
            elif do_mm:
                for t in range(Kn):
                    emit_grams(t)
            if not (do_mm and do_extract and do_math):
                nc.vector.memset(out_t[:], 0.0)
            nc.sync.dma_start(out=out_d[:], in_=out_t[:])
    return nc


# ---------------------------------------------------------------------------
# Host side
# ---------------------------------------------------------------------------
def plan_shards(num_atoms, n_classes=K):
    """Sort rows into 32 global tiles of 128; snake-assign 4 tiles per core.

    Returns (order, assign, core_chunks): assign[c] = 4 global tile indices
    (processed big-first), core_chunks[c] = matching chunk counts.
    """
    B = num_atoms.shape[0]
    ntiles = B // ROWS
    assert ntiles == N_CORES * n_classes
    order = np.argsort(num_atoms, kind="stable")
    nas = num_atoms[order]
    tile_chunks = [
        int((int(nas[(i + 1) * ROWS - 1]) + CHUNK - 1) // CHUNK)
        for i in range(ntiles)
    ]
    assign = []
    core_chunks = []
    for c in range(N_CORES):
        tiles = [c, 15 - c, 16 + c, 31 - c]
        tiles.sort(key=lambda t: -tile_chunks[t])  # big-first
        assign.append(tiles)
        core_chunks.append([tile_chunks[t] for t in tiles])
    return order, assign, core_chunks


def _pack_tile(x, y, na, Ct):
    """x, y: [128, nmax, 3] f32 (row-major positions), na: [128] int.
    Returns op [128, Ct, GROUPS, GW] f32 with atoms on dim 0 (partitions)."""
    nmax = x.shape[1]
    cap = Ct * CHUNK
    # data [b, n, 7]
    d = np.zeros((ROWS, cap, CW), np.float32)
    ncl = min(cap, nmax)
    d[:, :ncl, 0:3] = x[:, :ncl, :]
    d[:, :ncl, 3:6] = y[:, :ncl, :]
    mask = (np.arange(cap)[None, :] < na[:, None]).astype(np.float32)
    d[:, :, 0:6] *= mask[:, :, None]
    d[:, :, 6] = 1.0
    # op[p, g, c, 7r+k] = d[8r+g, c*128+p, k]   (group-major for strip loads)
    d = d.reshape(ROWS, Ct, CHUNK, CW)            # [b, c, p, k]
    d = d.transpose(2, 1, 0, 3)                   # [p, c, b, k]
    d = d.reshape(CHUNK, Ct, R, GROUPS, CW)       # [p, c, r, g, k]  (b = 8r+g)
    d = d.transpose(0, 3, 1, 2, 4)                # [p, g, c, r, k]
    return np.ascontiguousarray(d.reshape(CHUNK, GROUPS, Ct, GW))


def _op_np_dtype():
    return mybir.dt.np(OP_DT)


def shard_inputs(coords_input, coords_target, num_atoms, order, assign, core_chunks):
    import ml_dtypes

    B, ncols = coords_input.shape
    nmax = ncols // 3
    sel = np.zeros((GW, R * CW), np.float32)
    for j in range(R * CW):
        sel[j, j] = 1.0
    sel = sel.astype(ml_dtypes.bfloat16)
    in_maps = []
    core_row_idx = []
    for c in range(N_CORES):
        m = {"sel": sel}
        idx_all = []
        Kn = len(assign[c])
        meta = np.zeros((ROWS, Kn), np.float32)
        for t in range(Kn):
            gt = assign[c][t]
            idx = order[gt * ROWS : (gt + 1) * ROWS]
            idx_all.append(idx)
            na = num_atoms[idx]
            meta[:, t] = na.astype(np.float32)
            x = coords_input[idx].reshape(ROWS, nmax, 3)
            y = coords_target[idx].reshape(ROWS, nmax, 3)
            op = _pack_tile(x, y, na, core_chunks[c][t])
            m[f"op{t}"] = np.ascontiguousarray(
                op.reshape(CHUNK, -1)
            ).astype(_op_np_dtype())
        m["meta"] = meta
        in_maps.append(m)
        core_row_idx.append(np.concatenate(idx_all))
    return in_maps, core_row_idx


def unshard_outputs(results, core_row_idx, B):
    out = np.empty(B, dtype=np.float32)
    for c in range(N_CORES):
        o = results[c]["out"]  # [ROWS, K]
        out[core_row_idx[c]] = o.T.reshape(-1)
    return out


# ---------------------------------------------------------------------------
# Entry point
# ---------------------------------------------------------------------------
_PROG_CACHE = {}


def _get_program(chunks):
    key = tuple(chunks)
    if key not in _PROG_CACHE:
        _PROG_CACHE[key] = build_program(list(chunks))
    return _PROG_CACHE[key]


def kernel(coords_input, coords_target, num_atoms):
    from concourse.bass_utils import run_bass_kernel_spmd

    x = np.ascontiguousarray(np.asarray(coords_input, dtype=np.float32))
    y = np.ascontiguousarray(np.asarray(coords_target, dtype=np.float32))
    na = np.asarray(num_atoms).astype(np.int64)
    B, ncols = x.shape
    Kn = B // (N_CORES * ROWS)
    assert B == N_CORES * ROWS * Kn, f"unsupported batch {B}"

    order, assign, core_chunks = plan_shards(na, n_classes=Kn)
    in_maps, core_row_idx = shard_inputs(x, y, na, order, assign, core_chunks)
    # group cores by identical chunk tuples -> one program per group
    groups = {}
    for c in range(N_CORES):
        groups.setdefault(tuple(core_chunks[c]), []).append(c)
    results = [None] * N_CORES
    for chunks, cores in groups.items():
        nc = _get_program(chunks)
        res = run_bass_kernel_spmd(
            nc, [in_maps[c] for c in cores], core_ids=list(range(len(cores)))
        )
        for i, c in enumerate(cores):
            results[c] = res.results[i]
    out = unshard_outputs(results, core_row_idx, B)
    return out.astype(np.float32)
